# revision 24
# baseline (speedup 1.0000x reference)
"""Trainium2 Bass kernel for nn_DiffusioUnpool (GNN message passing).

Math: out = P @ z where P = D^-1/2 (A_e + I) D^-1/2, z = scatter(fea|atte, perm),
rewritten as segment-sum SpMM:
    deg[i]  = 1 + sum_{e: src=i} attr[e]
    dis     = rsqrt(deg)
    zs_k[k] = dis[perm[k]] * [fea[k], atte[k], 0pad]      (compact kept-node table)
    out[i]  = dis[i] * sum_{e: src=i, kept(dst)} attr[e] * zs_k[rank(dst)]
(self-loops folded in as edges with attr=1 and dst=i for kept i)

Sharding: row-shard across 8 cores (core c owns rows [1024c, 1024c+1024));
edges bucketed by src owner. Per core: all-graph degrees via one tensor_reduce
over a replicated row-major attr layout (an ncfw AllGather measured ~70us
FIXED latency regardless of payload, so replicating 1.3MB + one reduce is far
cheaper than communicating), locally built dis gather-table, dma_gather of
kept-node rows (4 SWDGE queues, single_packet=False), one-hot matmul
segment-sum into PSUM per 128-row window, final dis scaling.

dma_scatter_add is NOT used for accumulation: measured on HW, duplicate
indices within one call lose updates (only ~2 of 8 same-position duplicate
contributions land), so edge reductions go through PE one-hot matmuls and
the degree reduction uses a row-major attr layout + one tensor_reduce.

Numerics: f32 end-to-end (measured rel err ~3e-7 vs the f32 reference);
dis = reciprocal + sqrt + two Newton steps. A bf16 propagation path exists
(PROP_BF16=1, rel err ~2e-3) but is not faster: the span is bound by the
gather descriptor generation and serial chain latency more than compute
bytes (bf16 measured 131us vs 146us f32; not worth the error).
"""
import os as _os

import ml_dtypes
import numpy as np

import concourse.bacc as bacc
import concourse.mybir as mybir
import concourse.tile as tile
from concourse.bass_utils import run_bass_kernel_spmd
from concourse.tile import add_dep_helper
from concourse.library_config import mlp
from bass_rust import SyncInfo

F32 = mybir.dt.float32
BF16 = mybir.dt.bfloat16
I16 = mybir.dt.int16

NCORES = 8
NQUEUES = int(_os.environ.get("NQUEUES", "4"))
PROP_BF16 = _os.environ.get("PROP_BF16", "0") == "1"
D_FEAT = 128
DO = 129  # meaningful output row width: fea(128) | atte(1)


def _dp():
    # gatherable zs_k row width: 512B in bf16, 768B in f32 (256B-multiple rule)
    return 256 if PROP_BF16 else 192


def _split_multi_waits(nc):
    """This walrus build only encodes one sem-wait per instruction; hoist
    extras into wait-only EventSemaphore instructions just before."""
    for f in nc.m.functions:
        for bb in f.blocks:
            out = []
            changed = False
            for ins in bb.instructions:
                si = ins.sync_info
                if si is not None and si.on_wait is not None and len(si.on_wait) > 1:
                    waits = list(si.on_wait)
                    for k, w in enumerate(waits[:-1]):
                        ev = mybir.InstEventSemaphore(
                            name=f"{ins.name}-xw{k}", ins=[], outs=[]
                        )
                        ev.engine = ins.engine
                        ev.sync_info = SyncInfo(on_wait=[w], on_update=[])
                        out.append(ev)
                    si.on_wait = waits[-1:]
                    ins.sync_info = si
                    changed = True
                out.append(ins)
            if changed:
                bb.instructions = out


def _wrap_idx(idx):
    """[n] -> [128, n/16] int16: idx[i] at [i%16, i//16], replicated x8."""
    a = np.asarray(idx, np.int16).reshape(-1, 16).T
    return np.ascontiguousarray(np.tile(a, (8, 1)))


def _pack_windows(edge_lists, kw_list):
    """edge_lists: per window, (srcrel, attr, dstrank) arrays.
    Returns srcrel [128, C], attr [128, C], dstrank flat [C*128] with
    position (chunk, partition) = edge chunk*128+partition, windows
    concatenated chunk-major; padding edges srcrel=0/attr=0/dst=0."""
    C = sum(kw_list)
    srcrel = np.zeros((128, C), np.float32)
    attr = np.zeros((128, C), np.float32)
    dstr = np.zeros(C * 128, np.int64)
    col = 0
    for (sr, at, dr), kw in zip(edge_lists, kw_list, strict=True):
        m = len(sr)
        b = np.zeros(kw * 128, np.float32)
        b[:m] = sr
        srcrel[:, col : col + kw] = b.reshape(kw, 128).T
        b = np.zeros(kw * 128, np.float32)
        b[:m] = at
        attr[:, col : col + kw] = b.reshape(kw, 128).T
        b = np.zeros(kw * 128, np.int64)
        b[:m] = dr
        dstr[col * 128 : (col + kw) * 128] = b
        col += kw
    return srcrel, attr, dstr


def _prep(fea, perm, eidx, eattr, natte, n, ncores):
    """Host-side sharding/index prep. Numeric compute stays on device."""
    N = int(n)
    K = perm.shape[0]
    R = N // ncores
    W = R // 128
    src = eidx[0].astype(np.int64)
    dst = eidx[1].astype(np.int64)
    attr = eattr.astype(np.float32)

    kept = np.zeros(N, bool)
    kept[perm] = True
    rank = np.zeros(N, np.int64)
    rank[perm] = np.arange(K)

    # prop edge list: kept-dst edges + self edges (attr=1) for kept nodes
    keep_e = kept[dst]
    psrc = np.concatenate([src[keep_e], perm.astype(np.int64)])
    pdst = np.concatenate([rank[dst[keep_e]], rank[perm]])
    pattr = np.concatenate([attr[keep_e], np.ones(K, np.float32)])

    def bucket(s, a, dr):
        """-> per (core, window) edge arrays + shared chunk counts."""
        g = s // 128  # global window id
        order = np.argsort(g, kind="stable")
        s, a, g = s[order], a[order], g[order]
        dr = dr[order] if dr is not None else None
        counts = np.bincount(g, minlength=W * ncores)
        kw = np.maximum(
            1, -(-counts.reshape(ncores, W).max(axis=0) // 128)
        )  # [W] shared chunk counts
        offs = np.concatenate([[0], np.cumsum(counts)])
        per_core = []
        for c in range(ncores):
            lists = []
            for w in range(W):
                gi = c * W + w
                sl = slice(offs[gi], offs[gi + 1])
                lists.append(
                    (
                        (s[sl] % 128).astype(np.float32),
                        a[sl],
                        dr[sl]
                        if dr is not None
                        else np.zeros(offs[gi + 1] - offs[gi], np.int64),
                    )
                )
            per_core.append(_pack_windows(lists, kw))
        return per_core, list(int(x) for x in kw)

    prop_per_core, kprop = bucket(psrc, pattr, pdst)

    # deg: row-major packing — row r's edge attrs along the free dim of
    # partition r%128 (one tensor_reduce computes all degrees)
    order = np.argsort(src, kind="stable")
    ssrc = src[order]
    sattr = attr[order]
    counts = np.bincount(ssrc, minlength=N)
    MD = max(4, int(-(-counts.max() // 4) * 4))
    starts = np.concatenate([[0], np.cumsum(counts)])
    pos = np.arange(len(ssrc)) - starts[ssrc]
    byrow = np.zeros((N, MD), np.float32)
    byrow[ssrc, pos] = sattr

    WG = N // 128
    dga_full = np.ascontiguousarray(byrow.reshape(WG, 128, MD).transpose(1, 0, 2))

    dt_prop = ml_dtypes.bfloat16 if PROP_BF16 else np.float32
    in_maps = []
    for c in range(ncores):
        psr, pat, pdr = prop_per_core[c]
        dga = (
            byrow[c * R : (c + 1) * R]
            .reshape(W, 128, MD)
            .transpose(1, 0, 2)
        )
        in_maps.append(
            {
                "dgaf": dga_full,
                "dga": np.ascontiguousarray(dga),
                "psr": psr.astype(dt_prop),
                "pat": pat.astype(dt_prop),
                "pix": _wrap_idx(pdr),
                "kix": _wrap_idx(perm.astype(np.int64)),
                "fea": np.ascontiguousarray(fea.astype(np.float32)),
                "nat": np.ascontiguousarray(natte.astype(np.float32)),
            }
        )
    return in_maps, MD, kprop, K, R, W, N


def _build(N, K, R, W, MD, kprop, skip=frozenset()):
    CProp = sum(kprop)
    KCH = K // 128
    DP = _dp()
    TPROP = BF16 if PROP_BF16 else F32

    nc = bacc.Bacc(
        "TRN2",
        target_bir_lowering=False,
        debug=False,
        num_devices=NCORES,
        num_swdge_queues=NQUEUES,
    )

    WG = N // 128
    dgaf_d = nc.dram_tensor("dgaf", [128, WG, MD], F32, kind="ExternalInput")
    dga_d = nc.dram_tensor("dga", [128, W, MD], F32, kind="ExternalInput")
    psr_d = nc.dram_tensor("psr", [128, CProp], TPROP, kind="ExternalInput")
    pat_d = nc.dram_tensor("pat", [128, CProp], TPROP, kind="ExternalInput")
    pix_d = nc.dram_tensor("pix", [128, CProp * 8], I16, kind="ExternalInput")
    kix_d = nc.dram_tensor("kix", [128, K // 16], I16, kind="ExternalInput")
    fea_d = nc.dram_tensor("fea", [K, D_FEAT], F32, kind="ExternalInput")
    nat_d = nc.dram_tensor("nat", [K, 1], F32, kind="ExternalInput")
    out_d = nc.dram_tensor("out", [R, DO], F32, kind="ExternalOutput")

    dpf_d = nc.dram_tensor("dpf", [N, 64], F32)  # locally built dis table
    zsk_d = nc.dram_tensor("zsk", [K, DP], TPROP)  # gather table

    with tile.TileContext(nc) as tc:
        with (
            tc.tile_pool(name="cst", bufs=1) as cst,
            tc.tile_pool(name="sprop", bufs=W) as sprop,
            tc.tile_pool(name="gwin", bufs=4) as gwin,
            tc.tile_pool(name="ps", bufs=2, space="PSUM") as ps,
        ):
            nc.gpsimd.load_library(mlp)

            iota_prop = cst.tile([128, 128], TPROP)
            nc.gpsimd.iota(
                iota_prop[:], [[1, 128]], channel_multiplier=0,
                allow_small_or_imprecise_dtypes=True,
            )

            # dis-table tile zeroed up front (no deps -> overlaps input DMAs)
            dpt = cst.tile([128, WG, 64], F32)
            nc.gpsimd.memset(dpt[:].rearrange("p a d -> p (a d)"), 0.0)

            dgaf_t = cst.tile([128, WG, MD], F32)
            dga_t = cst.tile([128, W, MD], F32)
            psr_t = cst.tile([128, CProp], TPROP)
            pat_t = cst.tile([128, CProp], TPROP)
            pix_t = cst.tile([128, CProp * 8], I16)
            kix_t = cst.tile([128, K // 16], I16)
            nc.sync.dma_start(dgaf_t[:], dgaf_d[:])
            nc.sync.dma_start(dga_t[:], dga_d[:])
            nc.sync.dma_start(kix_t[:], kix_d[:])
            nc.sync.dma_start(psr_t[:], psr_d[:])
            nc.sync.dma_start(pat_t[:], pat_d[:])

            fea_t = cst.tile([128, KCH, D_FEAT], F32)
            nc.scalar.dma_start(fea_t[:], fea_d[:].rearrange("(a p) d -> p a d", p=128))
            nat_t = cst.tile([128, KCH], F32)
            nc.scalar.dma_start(
                nat_t[:], nat_d[:].rearrange("(a p) one -> p (a one)", p=128)
            )
            nc.sync.dma_start(pix_t[:], pix_d[:])

            # ---- phase 1+2: degrees for the WHOLE graph, replicated on every
            # core (1.3MB input + one reduce beats a ~70us-fixed-latency
            # AllGather), plus a local copy for the final scaling ----
            def rsqrt_chain(x_wide, width):
                degp = cst.tile([128, width], F32, tag=f"degp{width}")
                nc.vector.tensor_scalar_add(degp[:], x_wide, 1.0)
                dis = cst.tile([128, width], F32, tag=f"dis{width}")
                nc.vector.reciprocal(dis[:], degp[:])
                nc.scalar.activation(
                    dis[:], dis[:], mybir.ActivationFunctionType.Sqrt
                )
                tmp = cst.tile([128, width], F32, tag=f"tmp{width}")
                for _ in range(2):
                    nc.vector.tensor_mul(tmp[:], dis[:], dis[:])
                    nc.vector.tensor_mul(tmp[:], tmp[:], degp[:])
                    nc.vector.tensor_scalar(
                        tmp[:], tmp[:], -0.5, 1.5,
                        mybir.AluOpType.mult, mybir.AluOpType.add,
                    )
                    nc.vector.tensor_mul(dis[:], dis[:], tmp[:])
                return dis

            degf_t = cst.tile([128, WG], F32)
            nc.vector.tensor_reduce(
                degf_t[:].unsqueeze(-1), dgaf_t[:], mybir.AxisListType.X,
                mybir.AluOpType.add,
            )
            disf_t = rsqrt_chain(degf_t[:], WG)

            deg_t = cst.tile([128, W], F32)
            nc.vector.tensor_reduce(
                deg_t[:].unsqueeze(-1), dga_t[:], mybir.AxisListType.X,
                mybir.AluOpType.add,
            )
            dis_t = rsqrt_chain(deg_t[:], W)

            # dis table [N, 64] written locally; only col 0 is ever gathered
            dpt_cp = nc.vector.tensor_copy(dpt[:, :, 0:1], disf_t[:].unsqueeze(-1))
            nc.scalar.dma_start(dpf_d[:].rearrange("(g p) e -> p g e", p=128), dpt[:])

            # ---- prop one-hots: emitted after the collective so the tiny
            # deg->dis chain wins the Vector stream; these overlap the
            # collective wait and the dis-table gather ----
            s_props = []
            off = 0
            for w in range(W):
                kw = kprop[w]
                S = sprop.tile([128, kw, 128], TPROP, tag="S")
                s_eq = nc.vector.tensor_tensor(
                    S[:],
                    iota_prop[:].unsqueeze(1).broadcast_to([128, kw, 128]),
                    psr_t[:, off : off + kw].unsqueeze(-1).broadcast_to([128, kw, 128]),
                    mybir.AluOpType.is_equal,
                )
                # keep the deg->dis-table chain first in the DVE stream: the
                # one-hot builds are bulky and only needed by the matmuls
                add_dep_helper(
                    dpt_cp.ins, s_eq.ins, sync=False,
                    reason="S builds after dis-table copy",
                )
                nc.vector.tensor_tensor(
                    S[:],
                    S[:],
                    pat_t[:, off : off + kw].unsqueeze(-1).broadcast_to([128, kw, 128]),
                    mybir.AluOpType.mult,
                )
                s_props.append(S)
                off += kw

            # ---- phase 3: zs_k table ----
            disk = cst.tile([128, KCH, 64], F32)
            if "kgather" in skip:
                nc.vector.memset(disk[:].rearrange("p a d -> p (a d)"), 0.25)
            else:
                kq = KCH // NQUEUES
                for q in range(NQUEUES):
                    nc.gpsimd.dma_gather(
                        disk[:, q * kq : (q + 1) * kq, :], dpf_d[:],
                        kix_t[:, q * kq * 8 : (q + 1) * kq * 8],
                        kq * 128, kq * 128, 64,
                        single_packet=False, queue_num=q,
                    )
            # pad cols zeroed early (cheap, overlaps the deg/collective phase)
            zsk_t = cst.tile([128, KCH, DP], TPROP)
            nc.gpsimd.memset(zsk_t[:].rearrange("p a d -> p (a d)"), 0.0)
            nc.vector.tensor_tensor(
                zsk_t[:, :, 0:D_FEAT],
                fea_t[:],
                disk[:, :, 0:1].broadcast_to([128, KCH, D_FEAT]),
                mybir.AluOpType.mult,
            )
            nc.vector.tensor_tensor(
                zsk_t[:, :, D_FEAT : D_FEAT + 1],
                nat_t[:].unsqueeze(-1),
                disk[:, :, 0:1],
                mybir.AluOpType.mult,
            )
            nc.scalar.dma_start(zsk_d[:].rearrange("(a p) d -> p a d", p=128), zsk_t[:])

            # ---- phase 4: propagate per window ----
            off = 0
            for w in range(W):
                kw = kprop[w]
                S = s_props[w]
                g = gwin.tile([128, kw, DP], TPROP, tag="g")
                if "ggather" in skip:
                    nc.vector.memset(g[:].rearrange("p a d -> p (a d)"), 0.5)
                else:
                    nc.gpsimd.dma_gather(
                        g[:], zsk_d[:], pix_t[:, off * 8 : (off + kw) * 8],
                        kw * 128, kw * 128, DP, single_packet=False,
                        queue_num=w % NQUEUES,
                    )
                py = ps.tile([128, DO], F32, tag="py")
                for k in range(kw):
                    nc.tensor.matmul(
                        py[:],
                        S[:, k, :],
                        g[:, k, 0:DO],
                        start=(k == 0),
                        stop=(k == kw - 1),
                    )
                ot = gwin.tile([128, DO], F32, tag="ot")
                nc.vector.tensor_scalar(
                    ot[:], py[:], dis_t[:, w : w + 1], None, mybir.AluOpType.mult
                )
                nc.sync.dma_start(
                    out_d[:].rearrange("(w p) d -> p w d", p=128)[:, w, :], ot[:]
                )
                off += kw

    nc.compile()
    return nc


_CACHE = {}


def kernel(fea, perm, encoder_edge_index, encoder_edge_attr, node_atte_coffe, node_num):
    fea = np.asarray(fea)
    perm = np.asarray(perm)
    eidx = np.asarray(encoder_edge_index)
    eattr = np.asarray(encoder_edge_attr)
    natte = np.asarray(node_atte_coffe)
    n = int(node_num)

    in_maps, MD, kprop, K, R, W, N = _prep(fea, perm, eidx, eattr, natte, n, NCORES)

    key = (N, K, MD, tuple(kprop))
    if key not in _CACHE:
        nc = _build(N, K, R, W, MD, kprop)
        _split_multi_waits(nc)
        _CACHE[key] = nc
    nc = _CACHE[key]

    res = run_bass_kernel_spmd(nc, in_maps, core_ids=list(range(NCORES)))
    full = np.concatenate([res.results[c]["out"] for c in range(NCORES)], axis=0)
    return full[:, :D_FEAT], full[:, D_FEAT:DO]


# revision 25
# speedup vs baseline: 1.1383x; 1.1383x over previous
"""Trainium2 Bass kernel for nn_DiffusioUnpool (GNN message passing).

Math: out = P @ z where P = D^-1/2 (A_e + I) D^-1/2, z = scatter(fea|atte, perm),
rewritten as segment-sum SpMM:
    deg[i]  = 1 + sum_{e: src=i} attr[e]
    dis     = rsqrt(deg)
    zs_k[k] = dis[perm[k]] * [fea[k], atte[k], 0pad]      (compact kept-node table)
    out[i]  = dis[i] * sum_{e: src=i, kept(dst)} attr[e] * zs_k[rank(dst)]
(self-loops folded in as edges with attr=1 and dst=i for kept i)

Sharding: row-shard across 8 cores (core c owns rows [1024c, 1024c+1024));
edges bucketed by src owner. Per core: all-graph degrees via one tensor_reduce
over a replicated row-major attr layout (an ncfw AllGather measured ~70us
FIXED latency regardless of payload, so replicating 1.3MB + one reduce is far
cheaper than communicating), locally built dis gather-table, dma_gather of
kept-node rows (4 SWDGE queues, single_packet=False), one-hot matmul
segment-sum into PSUM per 128-row window, final dis scaling.

dma_scatter_add is NOT used for accumulation: measured on HW, duplicate
indices within one call lose updates (only ~2 of 8 same-position duplicate
contributions land), so edge reductions go through PE one-hot matmuls and
the degree reduction uses a row-major attr layout + one tensor_reduce.

Numerics: f32 end-to-end (measured rel err ~3e-7 vs the f32 reference);
dis = reciprocal + sqrt + two Newton steps. A bf16 propagation path exists
(PROP_BF16=1, rel err ~2e-3) but is not faster: the span is bound by the
gather descriptor generation and serial chain latency more than compute
bytes (bf16 measured 131us vs 146us f32; not worth the error).
"""
import os as _os

import ml_dtypes
import numpy as np

import concourse.bacc as bacc
import concourse.mybir as mybir
import concourse.tile as tile
from concourse.bass_utils import run_bass_kernel_spmd
from concourse.library_config import mlp
from bass_rust import SyncInfo

F32 = mybir.dt.float32
BF16 = mybir.dt.bfloat16
I16 = mybir.dt.int16

NCORES = 8
NQUEUES = int(_os.environ.get("NQUEUES", "4"))
PROP_BF16 = _os.environ.get("PROP_BF16", "0") == "1"
D_FEAT = 128
DO = 129  # meaningful output row width: fea(128) | atte(1)


def _dp():
    # gatherable zs_k row width: 512B in bf16, 768B in f32 (256B-multiple rule)
    return 256 if PROP_BF16 else 192


def _split_multi_waits(nc):
    """This walrus build only encodes one sem-wait per instruction; hoist
    extras into wait-only EventSemaphore instructions just before."""
    for f in nc.m.functions:
        for bb in f.blocks:
            out = []
            changed = False
            for ins in bb.instructions:
                si = ins.sync_info
                if si is not None and si.on_wait is not None and len(si.on_wait) > 1:
                    waits = list(si.on_wait)
                    for k, w in enumerate(waits[:-1]):
                        ev = mybir.InstEventSemaphore(
                            name=f"{ins.name}-xw{k}", ins=[], outs=[]
                        )
                        ev.engine = ins.engine
                        ev.sync_info = SyncInfo(on_wait=[w], on_update=[])
                        out.append(ev)
                    si.on_wait = waits[-1:]
                    ins.sync_info = si
                    changed = True
                out.append(ins)
            if changed:
                bb.instructions = out


def _wrap_idx(idx):
    """[n] -> [128, n/16] int16: idx[i] at [i%16, i//16], replicated x8."""
    a = np.asarray(idx, np.int16).reshape(-1, 16).T
    return np.ascontiguousarray(np.tile(a, (8, 1)))


def _pack_windows(edge_lists, kw_list):
    """edge_lists: per window, (srcrel, attr, dstrank) arrays.
    Returns srcrel [128, C], attr [128, C], dstrank flat [C*128] with
    position (chunk, partition) = edge chunk*128+partition, windows
    concatenated chunk-major; padding edges srcrel=0/attr=0/dst=0."""
    C = sum(kw_list)
    srcrel = np.zeros((128, C), np.float32)
    attr = np.zeros((128, C), np.float32)
    dstr = np.zeros(C * 128, np.int64)
    col = 0
    for (sr, at, dr), kw in zip(edge_lists, kw_list, strict=True):
        m = len(sr)
        b = np.zeros(kw * 128, np.float32)
        b[:m] = sr
        srcrel[:, col : col + kw] = b.reshape(kw, 128).T
        b = np.zeros(kw * 128, np.float32)
        b[:m] = at
        attr[:, col : col + kw] = b.reshape(kw, 128).T
        b = np.zeros(kw * 128, np.int64)
        b[:m] = dr
        dstr[col * 128 : (col + kw) * 128] = b
        col += kw
    return srcrel, attr, dstr


def _prep(fea, perm, eidx, eattr, natte, n, ncores):
    """Host-side sharding/index prep. Numeric compute stays on device."""
    N = int(n)
    K = perm.shape[0]
    R = N // ncores
    W = R // 128
    src = eidx[0].astype(np.int64)
    dst = eidx[1].astype(np.int64)
    attr = eattr.astype(np.float32)

    kept = np.zeros(N, bool)
    kept[perm] = True
    rank = np.zeros(N, np.int64)
    rank[perm] = np.arange(K)

    # prop edge list: kept-dst edges + self edges (attr=1) for kept nodes
    keep_e = kept[dst]
    psrc = np.concatenate([src[keep_e], perm.astype(np.int64)])
    pdst = np.concatenate([rank[dst[keep_e]], rank[perm]])
    pattr = np.concatenate([attr[keep_e], np.ones(K, np.float32)])

    def bucket(s, a, dr):
        """-> per (core, window) edge arrays + shared chunk counts."""
        g = s // 128  # global window id
        order = np.argsort(g, kind="stable")
        s, a, g = s[order], a[order], g[order]
        dr = dr[order] if dr is not None else None
        counts = np.bincount(g, minlength=W * ncores)
        kw = np.maximum(
            1, -(-counts.reshape(ncores, W).max(axis=0) // 128)
        )  # [W] shared chunk counts
        offs = np.concatenate([[0], np.cumsum(counts)])
        per_core = []
        for c in range(ncores):
            lists = []
            for w in range(W):
                gi = c * W + w
                sl = slice(offs[gi], offs[gi + 1])
                lists.append(
                    (
                        (s[sl] % 128).astype(np.float32),
                        a[sl],
                        dr[sl]
                        if dr is not None
                        else np.zeros(offs[gi + 1] - offs[gi], np.int64),
                    )
                )
            per_core.append(_pack_windows(lists, kw))
        return per_core, list(int(x) for x in kw)

    prop_per_core, kprop = bucket(psrc, pattr, pdst)

    # deg: row-major packing — row r's edge attrs along the free dim of
    # partition r%128 (one tensor_reduce computes all degrees)
    order = np.argsort(src, kind="stable")
    ssrc = src[order]
    sattr = attr[order]
    counts = np.bincount(ssrc, minlength=N)
    MD = max(4, int(-(-counts.max() // 4) * 4))
    starts = np.concatenate([[0], np.cumsum(counts)])
    pos = np.arange(len(ssrc)) - starts[ssrc]
    byrow = np.zeros((N, MD), np.float32)
    byrow[ssrc, pos] = sattr

    WG = N // 128
    dga_full = np.ascontiguousarray(byrow.reshape(WG, 128, MD).transpose(1, 0, 2))

    dt_prop = ml_dtypes.bfloat16 if PROP_BF16 else np.float32
    in_maps = []
    for c in range(ncores):
        psr, pat, pdr = prop_per_core[c]
        dga = (
            byrow[c * R : (c + 1) * R]
            .reshape(W, 128, MD)
            .transpose(1, 0, 2)
        )
        in_maps.append(
            {
                "dgaf": dga_full,
                "dga": np.ascontiguousarray(dga),
                "psr": psr.astype(dt_prop),
                "pat": pat.astype(dt_prop),
                "pix": _wrap_idx(pdr),
                "kix": _wrap_idx(perm.astype(np.int64)),
                "fea": np.ascontiguousarray(fea.astype(np.float32)),
                "nat": np.ascontiguousarray(natte.astype(np.float32)),
            }
        )
    return in_maps, MD, kprop, K, R, W, N


def _build(N, K, R, W, MD, kprop, skip=frozenset()):
    CProp = sum(kprop)
    KCH = K // 128
    DP = _dp()
    TPROP = BF16 if PROP_BF16 else F32

    nc = bacc.Bacc(
        "TRN2",
        target_bir_lowering=False,
        debug=False,
        num_devices=NCORES,
        num_swdge_queues=NQUEUES,
    )

    WG = N // 128
    dgaf_d = nc.dram_tensor("dgaf", [128, WG, MD], F32, kind="ExternalInput")
    dga_d = nc.dram_tensor("dga", [128, W, MD], F32, kind="ExternalInput")
    psr_d = nc.dram_tensor("psr", [128, CProp], TPROP, kind="ExternalInput")
    pat_d = nc.dram_tensor("pat", [128, CProp], TPROP, kind="ExternalInput")
    pix_d = nc.dram_tensor("pix", [128, CProp * 8], I16, kind="ExternalInput")
    kix_d = nc.dram_tensor("kix", [128, K // 16], I16, kind="ExternalInput")
    fea_d = nc.dram_tensor("fea", [K, D_FEAT], F32, kind="ExternalInput")
    nat_d = nc.dram_tensor("nat", [K, 1], F32, kind="ExternalInput")
    out_d = nc.dram_tensor("out", [R, DO], F32, kind="ExternalOutput")

    dpf_d = nc.dram_tensor("dpf", [N, 64], F32)  # locally built dis table
    zsk_d = nc.dram_tensor("zsk", [K, DP], TPROP)  # gather table

    with tile.TileContext(nc) as tc:
        with (
            tc.tile_pool(name="cst", bufs=1) as cst,
            tc.tile_pool(name="sprop", bufs=W) as sprop,
            tc.tile_pool(name="gwin", bufs=4) as gwin,
            tc.tile_pool(name="ps", bufs=2, space="PSUM") as ps,
        ):
            nc.gpsimd.load_library(mlp)

            iota_prop = cst.tile([128, 128], TPROP)
            nc.gpsimd.iota(
                iota_prop[:], [[1, 128]], channel_multiplier=0,
                allow_small_or_imprecise_dtypes=True,
            )

            # dis-table tile zeroed up front (no deps -> overlaps input DMAs)
            dpt = cst.tile([128, WG, 64], F32)
            nc.gpsimd.memset(dpt[:].rearrange("p a d -> p (a d)"), 0.0)

            dgaf_t = cst.tile([128, WG, MD], F32)
            dga_t = cst.tile([128, W, MD], F32)
            psr_t = cst.tile([128, CProp], TPROP)
            pat_t = cst.tile([128, CProp], TPROP)
            pix_t = cst.tile([128, CProp * 8], I16)
            kix_t = cst.tile([128, K // 16], I16)
            nc.sync.dma_start(dgaf_t[:], dgaf_d[:])
            nc.sync.dma_start(dga_t[:], dga_d[:])
            nc.sync.dma_start(kix_t[:], kix_d[:])
            nc.sync.dma_start(psr_t[:], psr_d[:])
            nc.sync.dma_start(pat_t[:], pat_d[:])

            fea_t = cst.tile([128, KCH, D_FEAT], F32)
            nc.scalar.dma_start(fea_t[:], fea_d[:].rearrange("(a p) d -> p a d", p=128))
            nat_t = cst.tile([128, KCH], F32)
            nc.scalar.dma_start(
                nat_t[:], nat_d[:].rearrange("(a p) one -> p (a one)", p=128)
            )
            nc.sync.dma_start(pix_t[:], pix_d[:])

            # ---- phase 1+2: degrees for the WHOLE graph, replicated on every
            # core (1.3MB input + one reduce beats a ~70us-fixed-latency
            # AllGather), plus a local copy for the final scaling ----
            def rsqrt_chain(x_wide, width):
                degp = cst.tile([128, width], F32, tag=f"degp{width}")
                nc.vector.tensor_scalar_add(degp[:], x_wide, 1.0)
                dis = cst.tile([128, width], F32, tag=f"dis{width}")
                nc.vector.reciprocal(dis[:], degp[:])
                nc.scalar.activation(
                    dis[:], dis[:], mybir.ActivationFunctionType.Sqrt
                )
                tmp = cst.tile([128, width], F32, tag=f"tmp{width}")
                for _ in range(2):
                    nc.vector.tensor_mul(tmp[:], dis[:], dis[:])
                    nc.vector.tensor_mul(tmp[:], tmp[:], degp[:])
                    nc.vector.tensor_scalar(
                        tmp[:], tmp[:], -0.5, 1.5,
                        mybir.AluOpType.mult, mybir.AluOpType.add,
                    )
                    nc.vector.tensor_mul(dis[:], dis[:], tmp[:])
                return dis

            degf_t = cst.tile([128, WG], F32)
            nc.vector.tensor_reduce(
                degf_t[:].unsqueeze(-1), dgaf_t[:], mybir.AxisListType.X,
                mybir.AluOpType.add,
            )
            disf_t = rsqrt_chain(degf_t[:], WG)

            deg_t = cst.tile([128, W], F32)
            nc.vector.tensor_reduce(
                deg_t[:].unsqueeze(-1), dga_t[:], mybir.AxisListType.X,
                mybir.AluOpType.add,
            )
            dis_t = rsqrt_chain(deg_t[:], W)

            # dis table [N, 64] written locally; only col 0 is ever gathered
            nc.vector.tensor_copy(dpt[:, :, 0:1], disf_t[:].unsqueeze(-1))
            nc.scalar.dma_start(dpf_d[:].rearrange("(g p) e -> p g e", p=128), dpt[:])

            # ---- prop one-hots: emitted after the collective so the tiny
            # deg->dis chain wins the Vector stream; these overlap the
            # collective wait and the dis-table gather ----
            s_props = []
            off = 0
            for w in range(W):
                kw = kprop[w]
                S = sprop.tile([128, kw, 128], TPROP, tag="S")
                nc.vector.tensor_tensor(
                    S[:],
                    iota_prop[:].unsqueeze(1).broadcast_to([128, kw, 128]),
                    psr_t[:, off : off + kw].unsqueeze(-1).broadcast_to([128, kw, 128]),
                    mybir.AluOpType.is_equal,
                )
                nc.vector.tensor_tensor(
                    S[:],
                    S[:],
                    pat_t[:, off : off + kw].unsqueeze(-1).broadcast_to([128, kw, 128]),
                    mybir.AluOpType.mult,
                )
                s_props.append(S)
                off += kw

            # ---- phase 3: zs_k table ----
            disk = cst.tile([128, KCH, 64], F32)
            if "kgather" in skip:
                nc.vector.memset(disk[:].rearrange("p a d -> p (a d)"), 0.25)
            else:
                kq = KCH // NQUEUES
                for q in range(NQUEUES):
                    nc.gpsimd.dma_gather(
                        disk[:, q * kq : (q + 1) * kq, :], dpf_d[:],
                        kix_t[:, q * kq * 8 : (q + 1) * kq * 8],
                        kq * 128, kq * 128, 64,
                        single_packet=False, queue_num=q,
                    )
            # pad cols zeroed early (cheap, overlaps the deg/collective phase)
            zsk_t = cst.tile([128, KCH, DP], TPROP)
            nc.gpsimd.memset(zsk_t[:].rearrange("p a d -> p (a d)"), 0.0)
            nc.vector.tensor_tensor(
                zsk_t[:, :, 0:D_FEAT],
                fea_t[:],
                disk[:, :, 0:1].broadcast_to([128, KCH, D_FEAT]),
                mybir.AluOpType.mult,
            )
            nc.vector.tensor_tensor(
                zsk_t[:, :, D_FEAT : D_FEAT + 1],
                nat_t[:].unsqueeze(-1),
                disk[:, :, 0:1],
                mybir.AluOpType.mult,
            )
            nc.scalar.dma_start(zsk_d[:].rearrange("(a p) d -> p a d", p=128), zsk_t[:])

            # ---- phase 4: propagate per window ----
            off = 0
            for w in range(W):
                kw = kprop[w]
                S = s_props[w]
                g = gwin.tile([128, kw, DP], TPROP, tag="g")
                if "ggather" in skip:
                    nc.vector.memset(g[:].rearrange("p a d -> p (a d)"), 0.5)
                else:
                    nc.gpsimd.dma_gather(
                        g[:], zsk_d[:], pix_t[:, off * 8 : (off + kw) * 8],
                        kw * 128, kw * 128, DP, single_packet=False,
                        queue_num=w % NQUEUES,
                    )
                py = ps.tile([128, DO], F32, tag="py")
                for k in range(kw):
                    nc.tensor.matmul(
                        py[:],
                        S[:, k, :],
                        g[:, k, 0:DO],
                        start=(k == 0),
                        stop=(k == kw - 1),
                    )
                ot = gwin.tile([128, DO], F32, tag="ot")
                nc.vector.tensor_scalar(
                    ot[:], py[:], dis_t[:, w : w + 1], None, mybir.AluOpType.mult
                )
                nc.sync.dma_start(
                    out_d[:].rearrange("(w p) d -> p w d", p=128)[:, w, :], ot[:]
                )
                off += kw

    nc.compile()
    return nc


_CACHE = {}


def kernel(fea, perm, encoder_edge_index, encoder_edge_attr, node_atte_coffe, node_num):
    fea = np.asarray(fea)
    perm = np.asarray(perm)
    eidx = np.asarray(encoder_edge_index)
    eattr = np.asarray(encoder_edge_attr)
    natte = np.asarray(node_atte_coffe)
    n = int(node_num)

    in_maps, MD, kprop, K, R, W, N = _prep(fea, perm, eidx, eattr, natte, n, NCORES)

    key = (N, K, MD, tuple(kprop))
    if key not in _CACHE:
        nc = _build(N, K, R, W, MD, kprop)
        _split_multi_waits(nc)
        _CACHE[key] = nc
    nc = _CACHE[key]

    res = run_bass_kernel_spmd(nc, in_maps, core_ids=list(range(NCORES)))
    full = np.concatenate([res.results[c]["out"] for c in range(NCORES)], axis=0)
    return full[:, :D_FEAT], full[:, D_FEAT:DO]


# revision 26
# speedup vs baseline: 1.1952x; 1.0500x over previous
"""Trainium2 Bass kernel for nn_DiffusioUnpool (GNN message passing).

Math: out = P @ z where P = D^-1/2 (A_e + I) D^-1/2, z = scatter(fea|atte, perm),
rewritten as segment-sum SpMM:
    deg[i]  = 1 + sum_{e: src=i} attr[e]
    dis     = rsqrt(deg)
    zs_k[k] = dis[perm[k]] * [fea[k], atte[k], 0pad]      (compact kept-node table)
    out[i]  = dis[i] * sum_{e: src=i, kept(dst)} attr[e] * zs_k[rank(dst)]
(self-loops folded in as edges with attr=1 and dst=i for kept i)

Sharding: row-shard across 8 cores (core c owns rows [1024c, 1024c+1024));
edges bucketed by src owner. Per core: all-graph degrees via one tensor_reduce
over a replicated row-major attr layout (an ncfw AllGather measured ~70us
FIXED latency regardless of payload, so replicating 1.3MB + one reduce is far
cheaper than communicating), locally built dis gather-table, dma_gather of
kept-node rows (4 SWDGE queues, single_packet=False), one-hot matmul
segment-sum into PSUM per 128-row window, final dis scaling.

dma_scatter_add is NOT used for accumulation: measured on HW, duplicate
indices within one call lose updates (only ~2 of 8 same-position duplicate
contributions land), so edge reductions go through PE one-hot matmuls and
the degree reduction uses a row-major attr layout + one tensor_reduce.

Numerics: f32 end-to-end (measured rel err ~3e-7 vs the f32 reference);
dis = reciprocal + sqrt + two Newton steps. A bf16 propagation path exists
(PROP_BF16=1, rel err ~2e-3) but is not faster: the span is bound by the
gather descriptor generation and serial chain latency more than compute
bytes (bf16 measured 131us vs 146us f32; not worth the error).
"""
import os as _os

import ml_dtypes
import numpy as np

import concourse.bacc as bacc
import concourse.mybir as mybir
import concourse.tile as tile
from concourse.bass_utils import run_bass_kernel_spmd
from concourse.library_config import mlp
from bass_rust import SyncInfo

F32 = mybir.dt.float32
BF16 = mybir.dt.bfloat16
I16 = mybir.dt.int16

NCORES = 8
NQUEUES = int(_os.environ.get("NQUEUES", "4"))
PROP_BF16 = _os.environ.get("PROP_BF16", "0") == "1"
D_FEAT = 128
DO = 129  # meaningful output row width: fea(128) | atte(1)


def _dp():
    # gatherable zs_k row width: 512B in bf16, 768B in f32 (256B-multiple rule)
    return 256 if PROP_BF16 else 192


def _split_multi_waits(nc):
    """This walrus build only encodes one sem-wait per instruction; hoist
    extras into wait-only EventSemaphore instructions just before."""
    for f in nc.m.functions:
        for bb in f.blocks:
            out = []
            changed = False
            for ins in bb.instructions:
                si = ins.sync_info
                if si is not None and si.on_wait is not None and len(si.on_wait) > 1:
                    waits = list(si.on_wait)
                    for k, w in enumerate(waits[:-1]):
                        ev = mybir.InstEventSemaphore(
                            name=f"{ins.name}-xw{k}", ins=[], outs=[]
                        )
                        ev.engine = ins.engine
                        ev.sync_info = SyncInfo(on_wait=[w], on_update=[])
                        out.append(ev)
                    si.on_wait = waits[-1:]
                    ins.sync_info = si
                    changed = True
                out.append(ins)
            if changed:
                bb.instructions = out


def _wrap_idx(idx):
    """[n] -> [128, n/16] int16: idx[i] at [i%16, i//16], replicated x8."""
    a = np.asarray(idx, np.int16).reshape(-1, 16).T
    return np.ascontiguousarray(np.tile(a, (8, 1)))


def _pack_windows(edge_lists, kw_list):
    """edge_lists: per window, (srcrel, attr, dstrank) arrays.
    Returns srcrel [128, C], attr [128, C], dstrank flat [C*128] with
    position (chunk, partition) = edge chunk*128+partition, windows
    concatenated chunk-major; padding edges srcrel=0/attr=0/dst=0."""
    C = sum(kw_list)
    srcrel = np.zeros((128, C), np.float32)
    attr = np.zeros((128, C), np.float32)
    dstr = np.zeros(C * 128, np.int64)
    col = 0
    for (sr, at, dr), kw in zip(edge_lists, kw_list, strict=True):
        m = len(sr)
        b = np.zeros(kw * 128, np.float32)
        b[:m] = sr
        srcrel[:, col : col + kw] = b.reshape(kw, 128).T
        b = np.zeros(kw * 128, np.float32)
        b[:m] = at
        attr[:, col : col + kw] = b.reshape(kw, 128).T
        b = np.zeros(kw * 128, np.int64)
        b[:m] = dr
        dstr[col * 128 : (col + kw) * 128] = b
        col += kw
    return srcrel, attr, dstr


def _prep(fea, perm, eidx, eattr, natte, n, ncores):
    """Host-side sharding/index prep. Numeric compute stays on device."""
    N = int(n)
    K = perm.shape[0]
    R = N // ncores
    W = R // 128
    src = eidx[0].astype(np.int64)
    dst = eidx[1].astype(np.int64)
    attr = eattr.astype(np.float32)

    kept = np.zeros(N, bool)
    kept[perm] = True
    rank = np.zeros(N, np.int64)
    rank[perm] = np.arange(K)

    # prop edge list: kept-dst edges + self edges (attr=1) for kept nodes
    keep_e = kept[dst]
    psrc = np.concatenate([src[keep_e], perm.astype(np.int64)])
    pdst = np.concatenate([rank[dst[keep_e]], rank[perm]])
    pattr = np.concatenate([attr[keep_e], np.ones(K, np.float32)])

    def bucket(s, a, dr):
        """-> per (core, window) edge arrays + shared chunk counts."""
        g = s // 128  # global window id
        order = np.argsort(g, kind="stable")
        s, a, g = s[order], a[order], g[order]
        dr = dr[order] if dr is not None else None
        counts = np.bincount(g, minlength=W * ncores)
        kw = np.maximum(
            1, -(-counts.reshape(ncores, W).max(axis=0) // 128)
        )  # [W] shared chunk counts
        offs = np.concatenate([[0], np.cumsum(counts)])
        per_core = []
        for c in range(ncores):
            lists = []
            for w in range(W):
                gi = c * W + w
                sl = slice(offs[gi], offs[gi + 1])
                lists.append(
                    (
                        (s[sl] % 128).astype(np.float32),
                        a[sl],
                        dr[sl]
                        if dr is not None
                        else np.zeros(offs[gi + 1] - offs[gi], np.int64),
                    )
                )
            per_core.append(_pack_windows(lists, kw))
        return per_core, list(int(x) for x in kw)

    prop_per_core, kprop = bucket(psrc, pattr, pdst)

    # deg: row-major packing — row r's edge attrs along the free dim of
    # partition r%128 (one tensor_reduce computes all degrees)
    order = np.argsort(src, kind="stable")
    ssrc = src[order]
    sattr = attr[order]
    counts = np.bincount(ssrc, minlength=N)
    MD = max(4, int(-(-counts.max() // 4) * 4))
    starts = np.concatenate([[0], np.cumsum(counts)])
    pos = np.arange(len(ssrc)) - starts[ssrc]
    byrow = np.zeros((N, MD), np.float32)
    byrow[ssrc, pos] = sattr

    WG = N // 128
    dga_full = np.ascontiguousarray(byrow.reshape(WG, 128, MD).transpose(1, 0, 2))

    dt_prop = ml_dtypes.bfloat16 if PROP_BF16 else np.float32
    in_maps = []
    for c in range(ncores):
        psr, pat, pdr = prop_per_core[c]
        dga = (
            byrow[c * R : (c + 1) * R]
            .reshape(W, 128, MD)
            .transpose(1, 0, 2)
        )
        in_maps.append(
            {
                "dgaf": dga_full,
                "dga": np.ascontiguousarray(dga),
                "psr": psr.astype(dt_prop),
                "pat": pat.astype(dt_prop),
                "pix": _wrap_idx(pdr),
                "kix": _wrap_idx(perm.astype(np.int64)),
                "fea": np.ascontiguousarray(fea.astype(np.float32)),
                "nat": np.ascontiguousarray(natte.astype(np.float32)),
            }
        )
    return in_maps, MD, kprop, K, R, W, N


def _build(N, K, R, W, MD, kprop, skip=frozenset()):
    CProp = sum(kprop)
    KCH = K // 128
    DP = _dp()
    TPROP = BF16 if PROP_BF16 else F32

    nc = bacc.Bacc(
        "TRN2",
        target_bir_lowering=False,
        debug=False,
        num_devices=NCORES,
        num_swdge_queues=NQUEUES,
    )

    WG = N // 128
    dgaf_d = nc.dram_tensor("dgaf", [128, WG, MD], F32, kind="ExternalInput")
    dga_d = nc.dram_tensor("dga", [128, W, MD], F32, kind="ExternalInput")
    psr_d = nc.dram_tensor("psr", [128, CProp], TPROP, kind="ExternalInput")
    pat_d = nc.dram_tensor("pat", [128, CProp], TPROP, kind="ExternalInput")
    pix_d = nc.dram_tensor("pix", [128, CProp * 8], I16, kind="ExternalInput")
    kix_d = nc.dram_tensor("kix", [128, K // 16], I16, kind="ExternalInput")
    fea_d = nc.dram_tensor("fea", [K, D_FEAT], F32, kind="ExternalInput")
    nat_d = nc.dram_tensor("nat", [K, 1], F32, kind="ExternalInput")
    out_d = nc.dram_tensor("out", [R, DO], F32, kind="ExternalOutput")

    dpf_d = nc.dram_tensor("dpf", [N, 64], F32)  # locally built dis table
    zsk_d = nc.dram_tensor("zsk", [K, DP], TPROP)  # gather table

    with tile.TileContext(nc) as tc:
        with (
            tc.tile_pool(name="cst", bufs=1) as cst,
            tc.tile_pool(name="sprop", bufs=W) as sprop,
            tc.tile_pool(name="gwin", bufs=4) as gwin,
            tc.tile_pool(name="ps", bufs=2, space="PSUM") as ps,
        ):
            nc.gpsimd.load_library(mlp)

            iota_prop = cst.tile([128, 128], TPROP)
            nc.gpsimd.iota(
                iota_prop[:], [[1, 128]], channel_multiplier=0,
                allow_small_or_imprecise_dtypes=True,
            )

            # dis-table tile zeroed up front (no deps -> overlaps input DMAs)
            dpt = cst.tile([128, WG, 64], F32)
            nc.vector.memset(dpt[:].rearrange("p a d -> p (a d)"), 0.0)

            dgaf_t = cst.tile([128, WG, MD], F32)
            dga_t = cst.tile([128, W, MD], F32)
            psr_t = cst.tile([128, CProp], TPROP)
            pat_t = cst.tile([128, CProp], TPROP)
            pix_t = cst.tile([128, CProp * 8], I16)
            kix_t = cst.tile([128, K // 16], I16)
            nc.sync.dma_start(dgaf_t[:], dgaf_d[:])
            nc.sync.dma_start(dga_t[:], dga_d[:])
            nc.sync.dma_start(kix_t[:], kix_d[:])
            nc.sync.dma_start(psr_t[:], psr_d[:])
            nc.sync.dma_start(pat_t[:], pat_d[:])

            fea_t = cst.tile([128, KCH, D_FEAT], F32)
            nc.sync.dma_start(fea_t[:], fea_d[:].rearrange("(a p) d -> p a d", p=128))
            nat_t = cst.tile([128, KCH], F32)
            nc.sync.dma_start(
                nat_t[:], nat_d[:].rearrange("(a p) one -> p (a one)", p=128)
            )
            nc.sync.dma_start(pix_t[:], pix_d[:])

            # ---- phase 1+2: degrees for the WHOLE graph, replicated on every
            # core (1.3MB input + one reduce beats a ~70us-fixed-latency
            # AllGather), plus a local copy for the final scaling ----
            def rsqrt_chain(x_wide, width):
                degp = cst.tile([128, width], F32, tag=f"degp{width}")
                nc.vector.tensor_scalar_add(degp[:], x_wide, 1.0)
                dis = cst.tile([128, width], F32, tag=f"dis{width}")
                nc.vector.reciprocal(dis[:], degp[:])
                nc.scalar.activation(
                    dis[:], dis[:], mybir.ActivationFunctionType.Sqrt
                )
                tmp = cst.tile([128, width], F32, tag=f"tmp{width}")
                for _ in range(2):
                    nc.vector.tensor_mul(tmp[:], dis[:], dis[:])
                    nc.vector.tensor_mul(tmp[:], tmp[:], degp[:])
                    nc.vector.tensor_scalar(
                        tmp[:], tmp[:], -0.5, 1.5,
                        mybir.AluOpType.mult, mybir.AluOpType.add,
                    )
                    nc.vector.tensor_mul(dis[:], dis[:], tmp[:])
                return dis

            degf_t = cst.tile([128, WG], F32)
            nc.vector.tensor_reduce(
                degf_t[:].unsqueeze(-1), dgaf_t[:], mybir.AxisListType.X,
                mybir.AluOpType.add,
            )
            disf_t = rsqrt_chain(degf_t[:], WG)

            deg_t = cst.tile([128, W], F32)
            nc.vector.tensor_reduce(
                deg_t[:].unsqueeze(-1), dga_t[:], mybir.AxisListType.X,
                mybir.AluOpType.add,
            )
            dis_t = rsqrt_chain(deg_t[:], W)

            # dis table [N, 64] written locally; only col 0 is ever gathered
            nc.vector.tensor_copy(dpt[:, :, 0:1], disf_t[:].unsqueeze(-1))
            nc.scalar.dma_start(dpf_d[:].rearrange("(g p) e -> p g e", p=128), dpt[:])

            # ---- prop one-hots: emitted after the collective so the tiny
            # deg->dis chain wins the Vector stream; these overlap the
            # collective wait and the dis-table gather ----
            s_props = []
            off = 0
            for w in range(W):
                kw = kprop[w]
                S = sprop.tile([128, kw, 128], TPROP, tag="S")
                nc.vector.tensor_tensor(
                    S[:],
                    iota_prop[:].unsqueeze(1).broadcast_to([128, kw, 128]),
                    psr_t[:, off : off + kw].unsqueeze(-1).broadcast_to([128, kw, 128]),
                    mybir.AluOpType.is_equal,
                )
                nc.vector.tensor_tensor(
                    S[:],
                    S[:],
                    pat_t[:, off : off + kw].unsqueeze(-1).broadcast_to([128, kw, 128]),
                    mybir.AluOpType.mult,
                )
                s_props.append(S)
                off += kw

            # ---- phase 3: zs_k table ----
            disk = cst.tile([128, KCH, 64], F32)
            if "kgather" in skip:
                nc.vector.memset(disk[:].rearrange("p a d -> p (a d)"), 0.25)
            else:
                kq = KCH // NQUEUES
                for q in range(NQUEUES):
                    nc.gpsimd.dma_gather(
                        disk[:, q * kq : (q + 1) * kq, :], dpf_d[:],
                        kix_t[:, q * kq * 8 : (q + 1) * kq * 8],
                        kq * 128, kq * 128, 64,
                        single_packet=False, queue_num=q,
                    )
            # pad cols zeroed early (cheap, overlaps the deg/collective phase)
            zsk_t = cst.tile([128, KCH, DP], TPROP)
            nc.vector.memset(zsk_t[:].rearrange("p a d -> p (a d)"), 0.0)
            nc.vector.tensor_tensor(
                zsk_t[:, :, 0:D_FEAT],
                fea_t[:],
                disk[:, :, 0:1].broadcast_to([128, KCH, D_FEAT]),
                mybir.AluOpType.mult,
            )
            nc.vector.tensor_tensor(
                zsk_t[:, :, D_FEAT : D_FEAT + 1],
                nat_t[:].unsqueeze(-1),
                disk[:, :, 0:1],
                mybir.AluOpType.mult,
            )
            nc.scalar.dma_start(zsk_d[:].rearrange("(a p) d -> p a d", p=128), zsk_t[:])

            # ---- phase 4: propagate per window ----
            off = 0
            for w in range(W):
                kw = kprop[w]
                S = s_props[w]
                g = gwin.tile([128, kw, DP], TPROP, tag="g")
                if "ggather" in skip:
                    nc.vector.memset(g[:].rearrange("p a d -> p (a d)"), 0.5)
                else:
                    nc.gpsimd.dma_gather(
                        g[:], zsk_d[:], pix_t[:, off * 8 : (off + kw) * 8],
                        kw * 128, kw * 128, DP, single_packet=False,
                        queue_num=w % NQUEUES,
                    )
                py = ps.tile([128, DO], F32, tag="py")
                for k in range(kw):
                    nc.tensor.matmul(
                        py[:],
                        S[:, k, :],
                        g[:, k, 0:DO],
                        start=(k == 0),
                        stop=(k == kw - 1),
                    )
                ot = gwin.tile([128, DO], F32, tag="ot")
                nc.vector.tensor_scalar(
                    ot[:], py[:], dis_t[:, w : w + 1], None, mybir.AluOpType.mult
                )
                nc.sync.dma_start(
                    out_d[:].rearrange("(w p) d -> p w d", p=128)[:, w, :], ot[:]
                )
                off += kw

    nc.compile()
    return nc


_CACHE = {}


def kernel(fea, perm, encoder_edge_index, encoder_edge_attr, node_atte_coffe, node_num):
    fea = np.asarray(fea)
    perm = np.asarray(perm)
    eidx = np.asarray(encoder_edge_index)
    eattr = np.asarray(encoder_edge_attr)
    natte = np.asarray(node_atte_coffe)
    n = int(node_num)

    in_maps, MD, kprop, K, R, W, N = _prep(fea, perm, eidx, eattr, natte, n, NCORES)

    key = (N, K, MD, tuple(kprop))
    if key not in _CACHE:
        nc = _build(N, K, R, W, MD, kprop)
        _split_multi_waits(nc)
        _CACHE[key] = nc
    nc = _CACHE[key]

    res = run_bass_kernel_spmd(nc, in_maps, core_ids=list(range(NCORES)))
    full = np.concatenate([res.results[c]["out"] for c in range(NCORES)], axis=0)
    return full[:, :D_FEAT], full[:, D_FEAT:DO]


# revision 28
# speedup vs baseline: 1.2881x; 1.0778x over previous
"""Trainium2 Bass kernel for nn_DiffusioUnpool (GNN message passing).

Math: out = P @ z where P = D^-1/2 (A_e + I) D^-1/2, z = scatter(fea|atte, perm),
rewritten as segment-sum SpMM:
    deg[i]  = 1 + sum_{e: src=i} attr[e]
    dis     = rsqrt(deg)
    zs_k[k] = dis[perm[k]] * [fea[k], atte[k], 0pad]      (compact kept-node table)
    out[i]  = dis[i] * sum_{e: src=i, kept(dst)} attr[e] * zs_k[rank(dst)]
(self-loops folded in as edges with attr=1 and dst=i for kept i)

Sharding: row-shard across 8 cores (core c owns rows [1024c, 1024c+1024));
edges bucketed by src owner. Per core: all-graph degrees via one tensor_reduce
over a replicated row-major attr layout (an ncfw AllGather measured ~70us
FIXED latency regardless of payload, so replicating 1.3MB + one reduce is far
cheaper than communicating), locally built dis gather-table, dma_gather of
kept-node rows (4 SWDGE queues, single_packet=False), one-hot matmul
segment-sum into PSUM per 128-row window, final dis scaling.

dma_scatter_add is NOT used for accumulation: measured on HW, duplicate
indices within one call lose updates (only ~2 of 8 same-position duplicate
contributions land), so edge reductions go through PE one-hot matmuls and
the degree reduction uses a row-major attr layout + one tensor_reduce.

Numerics: f32 end-to-end (measured rel err ~3e-7 vs the f32 reference);
dis = reciprocal + sqrt + two Newton steps. A bf16 propagation path exists
(PROP_BF16=1, rel err ~2e-3) but is not faster: the span is bound by the
gather descriptor generation and serial chain latency more than compute
bytes (bf16 measured 131us vs 146us f32; not worth the error).
"""
import os as _os

import ml_dtypes
import numpy as np

import concourse.bacc as bacc
import concourse.mybir as mybir
import concourse.tile as tile
from concourse.bass_utils import run_bass_kernel_spmd
from concourse.library_config import mlp
from bass_rust import SyncInfo

F32 = mybir.dt.float32
BF16 = mybir.dt.bfloat16
I16 = mybir.dt.int16

NCORES = 8
NQUEUES = int(_os.environ.get("NQUEUES", "4"))
PROP_BF16 = _os.environ.get("PROP_BF16", "0") == "1"
D_FEAT = 128
DO = 129  # meaningful output row width: fea(128) | atte(1)


def _dp():
    # gatherable zs_k row width: 512B in bf16, 768B in f32 (256B-multiple rule)
    return 256 if PROP_BF16 else 192


def _split_multi_waits(nc):
    """This walrus build only encodes one sem-wait per instruction; hoist
    extras into wait-only EventSemaphore instructions just before."""
    for f in nc.m.functions:
        for bb in f.blocks:
            out = []
            changed = False
            for ins in bb.instructions:
                si = ins.sync_info
                if si is not None and si.on_wait is not None and len(si.on_wait) > 1:
                    waits = list(si.on_wait)
                    for k, w in enumerate(waits[:-1]):
                        ev = mybir.InstEventSemaphore(
                            name=f"{ins.name}-xw{k}", ins=[], outs=[]
                        )
                        ev.engine = ins.engine
                        ev.sync_info = SyncInfo(on_wait=[w], on_update=[])
                        out.append(ev)
                    si.on_wait = waits[-1:]
                    ins.sync_info = si
                    changed = True
                out.append(ins)
            if changed:
                bb.instructions = out


def _wrap_idx(idx):
    """[n] -> [128, n/16] int16: idx[i] at [i%16, i//16], replicated x8."""
    a = np.asarray(idx, np.int16).reshape(-1, 16).T
    return np.ascontiguousarray(np.tile(a, (8, 1)))


def _pack_windows(edge_lists, kw_list):
    """edge_lists: per window, (srcrel, attr, dstrank) arrays.
    Returns srcrel [128, C], attr [128, C], dstrank flat [C*128] with
    position (chunk, partition) = edge chunk*128+partition, windows
    concatenated chunk-major; padding edges srcrel=0/attr=0/dst=0."""
    C = sum(kw_list)
    srcrel = np.zeros((128, C), np.float32)
    attr = np.zeros((128, C), np.float32)
    dstr = np.zeros(C * 128, np.int64)
    col = 0
    for (sr, at, dr), kw in zip(edge_lists, kw_list, strict=True):
        m = len(sr)
        b = np.zeros(kw * 128, np.float32)
        b[:m] = sr
        srcrel[:, col : col + kw] = b.reshape(kw, 128).T
        b = np.zeros(kw * 128, np.float32)
        b[:m] = at
        attr[:, col : col + kw] = b.reshape(kw, 128).T
        b = np.zeros(kw * 128, np.int64)
        b[:m] = dr
        dstr[col * 128 : (col + kw) * 128] = b
        col += kw
    return srcrel, attr, dstr


def _prep(fea, perm, eidx, eattr, natte, n, ncores):
    """Host-side sharding/index prep. Numeric compute stays on device."""
    N = int(n)
    K = perm.shape[0]
    R = N // ncores
    W = R // 128
    src = eidx[0].astype(np.int64)
    dst = eidx[1].astype(np.int64)
    attr = eattr.astype(np.float32)

    kept = np.zeros(N, bool)
    kept[perm] = True
    rank = np.zeros(N, np.int64)
    rank[perm] = np.arange(K)

    # prop edge list: kept-dst edges + self edges (attr=1) for kept nodes
    keep_e = kept[dst]
    psrc = np.concatenate([src[keep_e], perm.astype(np.int64)])
    pdst = np.concatenate([rank[dst[keep_e]], rank[perm]])
    pattr = np.concatenate([attr[keep_e], np.ones(K, np.float32)])

    def bucket(s, a, dr):
        """-> per (core, window) edge arrays + shared chunk counts."""
        g = s // 128  # global window id
        order = np.argsort(g, kind="stable")
        s, a, g = s[order], a[order], g[order]
        dr = dr[order] if dr is not None else None
        counts = np.bincount(g, minlength=W * ncores)
        kw = np.maximum(
            1, -(-counts.reshape(ncores, W).max(axis=0) // 128)
        )  # [W] shared chunk counts
        offs = np.concatenate([[0], np.cumsum(counts)])
        per_core = []
        for c in range(ncores):
            lists = []
            for w in range(W):
                gi = c * W + w
                sl = slice(offs[gi], offs[gi + 1])
                lists.append(
                    (
                        (s[sl] % 128).astype(np.float32),
                        a[sl],
                        dr[sl]
                        if dr is not None
                        else np.zeros(offs[gi + 1] - offs[gi], np.int64),
                    )
                )
            per_core.append(_pack_windows(lists, kw))
        return per_core, list(int(x) for x in kw)

    prop_per_core, kprop = bucket(psrc, pattr, pdst)

    # deg: row-major packing — row r's edge attrs along the free dim of
    # partition r%128 (one tensor_reduce computes all degrees)
    order = np.argsort(src, kind="stable")
    ssrc = src[order]
    sattr = attr[order]
    counts = np.bincount(ssrc, minlength=N)
    MD = max(4, int(-(-counts.max() // 4) * 4))
    starts = np.concatenate([[0], np.cumsum(counts)])
    pos = np.arange(len(ssrc)) - starts[ssrc]
    byrow = np.zeros((N, MD), np.float32)
    byrow[ssrc, pos] = sattr

    WG = N // 128
    dga_full = np.ascontiguousarray(byrow.reshape(WG, 128, MD).transpose(1, 0, 2))

    dt_prop = ml_dtypes.bfloat16 if PROP_BF16 else np.float32
    in_maps = []
    for c in range(ncores):
        psr, pat, pdr = prop_per_core[c]
        dga = (
            byrow[c * R : (c + 1) * R]
            .reshape(W, 128, MD)
            .transpose(1, 0, 2)
        )
        in_maps.append(
            {
                "dgaf": dga_full,
                "dga": np.ascontiguousarray(dga),
                "psr": psr.astype(dt_prop),
                "pat": pat.astype(dt_prop),
                "pix": _wrap_idx(pdr),
                "kix": _wrap_idx(perm.astype(np.int64)),
                "fea": np.ascontiguousarray(fea.astype(np.float32)),
                "nat": np.ascontiguousarray(natte.astype(np.float32)),
            }
        )
    return in_maps, MD, kprop, K, R, W, N


def _build(N, K, R, W, MD, kprop, skip=frozenset()):
    CProp = sum(kprop)
    KCH = K // 128
    DP = _dp()
    TPROP = BF16 if PROP_BF16 else F32

    nc = bacc.Bacc(
        "TRN2",
        target_bir_lowering=False,
        debug=False,
        num_devices=NCORES,
        num_swdge_queues=NQUEUES,
    )

    WG = N // 128
    dgaf_d = nc.dram_tensor("dgaf", [128, WG, MD], F32, kind="ExternalInput")
    dga_d = nc.dram_tensor("dga", [128, W, MD], F32, kind="ExternalInput")
    psr_d = nc.dram_tensor("psr", [128, CProp], TPROP, kind="ExternalInput")
    pat_d = nc.dram_tensor("pat", [128, CProp], TPROP, kind="ExternalInput")
    pix_d = nc.dram_tensor("pix", [128, CProp * 8], I16, kind="ExternalInput")
    kix_d = nc.dram_tensor("kix", [128, K // 16], I16, kind="ExternalInput")
    fea_d = nc.dram_tensor("fea", [K, D_FEAT], F32, kind="ExternalInput")
    nat_d = nc.dram_tensor("nat", [K, 1], F32, kind="ExternalInput")
    out_d = nc.dram_tensor("out", [R, DO], F32, kind="ExternalOutput")

    dpf_d = nc.dram_tensor("dpf", [N, 64], F32)  # locally built dis table
    zsk_d = nc.dram_tensor("zsk", [K, DP], TPROP)  # gather table

    with tile.TileContext(nc) as tc:
        with (
            tc.tile_pool(name="cst", bufs=1) as cst,
            tc.tile_pool(name="sprop", bufs=W) as sprop,
            tc.tile_pool(name="gwin", bufs=4) as gwin,
            tc.tile_pool(name="ps", bufs=2, space="PSUM") as ps,
        ):
            nc.gpsimd.load_library(mlp)

            iota_prop = cst.tile([128, 128], TPROP)
            nc.gpsimd.iota(
                iota_prop[:], [[1, 128]], channel_multiplier=0,
                allow_small_or_imprecise_dtypes=True,
            )

            # dis-table tile zeroed up front (no deps -> overlaps input DMAs)
            dpt = cst.tile([128, WG, 64], F32)
            nc.vector.memset(dpt[:].rearrange("p a d -> p (a d)"), 0.0)

            dgaf_t = cst.tile([128, WG, MD], F32)
            dga_t = cst.tile([128, W, MD], F32)
            psr_t = cst.tile([128, CProp], TPROP)
            pat_t = cst.tile([128, CProp], TPROP)
            pix_t = cst.tile([128, CProp * 8], I16)
            kix_t = cst.tile([128, K // 16], I16)
            nc.sync.dma_start(dgaf_t[:], dgaf_d[:])
            nc.sync.dma_start(dga_t[:], dga_d[:])
            nc.sync.dma_start(kix_t[:], kix_d[:])
            nc.sync.dma_start(psr_t[:], psr_d[:])
            nc.sync.dma_start(pat_t[:], pat_d[:])

            fea_t = cst.tile([128, KCH, D_FEAT], F32)
            nc.sync.dma_start(fea_t[:], fea_d[:].rearrange("(a p) d -> p a d", p=128))
            nat_t = cst.tile([128, KCH], F32)
            nc.sync.dma_start(
                nat_t[:], nat_d[:].rearrange("(a p) one -> p (a one)", p=128)
            )
            nc.sync.dma_start(pix_t[:], pix_d[:])

            # ---- phase 1+2: degrees for the WHOLE graph, replicated on every
            # core (1.3MB input + one reduce beats a ~70us-fixed-latency
            # AllGather), plus a local copy for the final scaling ----
            def rsqrt_chain(x_wide, width):
                degp = cst.tile([128, width], F32, tag=f"degp{width}")
                nc.vector.tensor_scalar_add(degp[:], x_wide, 1.0)
                dis = cst.tile([128, width], F32, tag=f"dis{width}")
                nc.vector.reciprocal(dis[:], degp[:])
                nc.scalar.activation(
                    dis[:], dis[:], mybir.ActivationFunctionType.Sqrt
                )
                tmp = cst.tile([128, width], F32, tag=f"tmp{width}")
                for _ in range(2):
                    nc.vector.tensor_mul(tmp[:], dis[:], dis[:])
                    nc.vector.tensor_mul(tmp[:], tmp[:], degp[:])
                    nc.vector.tensor_scalar(
                        tmp[:], tmp[:], -0.5, 1.5,
                        mybir.AluOpType.mult, mybir.AluOpType.add,
                    )
                    nc.vector.tensor_mul(dis[:], dis[:], tmp[:])
                return dis

            degf_t = cst.tile([128, WG], F32)
            nc.vector.tensor_reduce(
                degf_t[:].unsqueeze(-1), dgaf_t[:], mybir.AxisListType.X,
                mybir.AluOpType.add,
            )
            disf_t = rsqrt_chain(degf_t[:], WG)

            deg_t = cst.tile([128, W], F32)
            nc.vector.tensor_reduce(
                deg_t[:].unsqueeze(-1), dga_t[:], mybir.AxisListType.X,
                mybir.AluOpType.add,
            )
            dis_t = rsqrt_chain(deg_t[:], W)

            # dis table [N, 64] written locally; only col 0 is ever gathered
            nc.vector.tensor_copy(dpt[:, :, 0:1], disf_t[:].unsqueeze(-1))
            nc.scalar.dma_start(dpf_d[:].rearrange("(g p) e -> p g e", p=128), dpt[:])

            # ---- prop one-hots: emitted after the collective so the tiny
            # deg->dis chain wins the Vector stream; these overlap the
            # collective wait and the dis-table gather ----
            s_props = []
            off = 0
            for w in range(W):
                kw = kprop[w]
                S = sprop.tile([128, kw, 128], TPROP, tag="S")
                nc.vector.tensor_tensor(
                    S[:],
                    iota_prop[:].unsqueeze(1).broadcast_to([128, kw, 128]),
                    psr_t[:, off : off + kw].unsqueeze(-1).broadcast_to([128, kw, 128]),
                    mybir.AluOpType.is_equal,
                )
                nc.vector.tensor_tensor(
                    S[:],
                    S[:],
                    pat_t[:, off : off + kw].unsqueeze(-1).broadcast_to([128, kw, 128]),
                    mybir.AluOpType.mult,
                )
                s_props.append(S)
                off += kw

            # ---- phase 3: zs_k table ----
            disk = cst.tile([128, KCH, 64], F32)
            if "kgather" in skip:
                nc.vector.memset(disk[:].rearrange("p a d -> p (a d)"), 0.25)
            else:
                kq = KCH // NQUEUES
                for q in range(NQUEUES):
                    nc.gpsimd.dma_gather(
                        disk[:, q * kq : (q + 1) * kq, :], dpf_d[:],
                        kix_t[:, q * kq * 8 : (q + 1) * kq * 8],
                        kq * 128, kq * 128, 64,
                        single_packet=False, queue_num=q,
                    )
            # pad cols zeroed early (cheap, overlaps the deg/collective phase)
            zsk_t = cst.tile([128, KCH, DP], TPROP)
            nc.vector.memset(zsk_t[:].rearrange("p a d -> p (a d)"), 0.0)
            nc.vector.tensor_tensor(
                zsk_t[:, :, 0:D_FEAT],
                fea_t[:],
                disk[:, :, 0:1].broadcast_to([128, KCH, D_FEAT]),
                mybir.AluOpType.mult,
            )
            nc.vector.tensor_tensor(
                zsk_t[:, :, D_FEAT : D_FEAT + 1],
                nat_t[:].unsqueeze(-1),
                disk[:, :, 0:1],
                mybir.AluOpType.mult,
            )
            nc.scalar.dma_start(zsk_d[:].rearrange("(a p) d -> p a d", p=128), zsk_t[:])

            # ---- phase 4: propagate per window ----
            off = 0
            for w in range(W):
                kw = kprop[w]
                S = s_props[w]
                g = gwin.tile([128, kw, DP], TPROP, tag="g")
                if "ggather" in skip:
                    nc.vector.memset(g[:].rearrange("p a d -> p (a d)"), 0.5)
                else:
                    nc.gpsimd.dma_gather(
                        g[:], zsk_d[:], pix_t[:, off * 8 : (off + kw) * 8],
                        kw * 128, kw * 128, DP, single_packet=False,
                        queue_num=w % NQUEUES,
                    )
                py = ps.tile([128, DO], F32, tag="py")
                for k in range(kw):
                    nc.tensor.matmul(
                        py[:],
                        S[:, k, :],
                        g[:, k, 0:DO],
                        start=(k == 0),
                        stop=(k == kw - 1),
                    )
                ot = gwin.tile([128, DO], F32, tag="ot")
                nc.vector.tensor_scalar(
                    ot[:], py[:], dis_t[:, w : w + 1], None, mybir.AluOpType.mult
                )
                nc.sync.dma_start(
                    out_d[:].rearrange("(w p) d -> p w d", p=128)[:, w, :], ot[:]
                )
                off += kw

    nc.compile()
    return nc


_CACHE = {}


def kernel(fea, perm, encoder_edge_index, encoder_edge_attr, node_atte_coffe, node_num):
    fea = np.asarray(fea)
    perm = np.asarray(perm)
    eidx = np.asarray(encoder_edge_index)
    eattr = np.asarray(encoder_edge_attr)
    natte = np.asarray(node_atte_coffe)
    n = int(node_num)

    in_maps, MD, kprop, K, R, W, N = _prep(fea, perm, eidx, eattr, natte, n, NCORES)

    key = (N, K, MD, tuple(kprop))
    if key not in _CACHE:
        nc = _build(N, K, R, W, MD, kprop)
        _split_multi_waits(nc)
        _CACHE[key] = nc
    nc = _CACHE[key]

    res = run_bass_kernel_spmd(nc, in_maps, core_ids=list(range(NCORES)))
    full = np.concatenate([res.results[c]["out"] for c in range(NCORES)], axis=0)
    return full[:, :D_FEAT], full[:, D_FEAT:DO]


# revision 29
# speedup vs baseline: 1.3392x; 1.0397x over previous
"""Trainium2 Bass kernel for nn_DiffusioUnpool (GNN message passing).

Math: out = P @ z where P = D^-1/2 (A_e + I) D^-1/2, z = scatter(fea|atte, perm),
rewritten as segment-sum SpMM:
    deg[i]  = 1 + sum_{e: src=i} attr[e]
    dis     = rsqrt(deg)
    zs_k[k] = dis[perm[k]] * [fea[k], atte[k], 0pad]      (compact kept-node table)
    out[i]  = dis[i] * sum_{e: src=i, kept(dst)} attr[e] * zs_k[rank(dst)]
(self-loops folded in as edges with attr=1 and dst=i for kept i)

Sharding: row-shard across 8 cores (core c owns rows [1024c, 1024c+1024));
edges bucketed by src owner. Per core: all-graph degrees via one tensor_reduce
over a replicated row-major attr layout (an ncfw AllGather measured ~70us
FIXED latency regardless of payload, so replicating 1.3MB + one reduce is far
cheaper than communicating), locally built dis gather-table, dma_gather of
kept-node rows (4 SWDGE queues, single_packet=False), one-hot matmul
segment-sum into PSUM per 128-row window, final dis scaling.

dma_scatter_add is NOT used for accumulation: measured on HW, duplicate
indices within one call lose updates (only ~2 of 8 same-position duplicate
contributions land), so edge reductions go through PE one-hot matmuls and
the degree reduction uses a row-major attr layout + one tensor_reduce.

Numerics: f32 end-to-end (measured rel err ~3e-7 vs the f32 reference);
dis = reciprocal + sqrt + two Newton steps. A bf16 propagation path exists
(PROP_BF16=1, rel err ~2e-3) but is not faster: the span is bound by the
gather descriptor generation and serial chain latency more than compute
bytes (bf16 measured 131us vs 146us f32; not worth the error).
"""
import os as _os

import ml_dtypes
import numpy as np

import concourse.bacc as bacc
import concourse.mybir as mybir
import concourse.tile as tile
from concourse.bass_utils import run_bass_kernel_spmd
from concourse.library_config import mlp
from bass_rust import SyncInfo

F32 = mybir.dt.float32
BF16 = mybir.dt.bfloat16
I16 = mybir.dt.int16

NCORES = 8
NQUEUES = int(_os.environ.get("NQUEUES", "4"))
PROP_BF16 = _os.environ.get("PROP_BF16", "0") == "1"
D_FEAT = 128
DO = 129  # meaningful output row width: fea(128) | atte(1)


def _dp():
    # gatherable zs_k row width: 512B in bf16, 768B in f32 (256B-multiple rule)
    return 256 if PROP_BF16 else 192


def _split_multi_waits(nc):
    """This walrus build only encodes one sem-wait per instruction; hoist
    extras into wait-only EventSemaphore instructions just before."""
    for f in nc.m.functions:
        for bb in f.blocks:
            out = []
            changed = False
            for ins in bb.instructions:
                si = ins.sync_info
                if si is not None and si.on_wait is not None and len(si.on_wait) > 1:
                    waits = list(si.on_wait)
                    for k, w in enumerate(waits[:-1]):
                        ev = mybir.InstEventSemaphore(
                            name=f"{ins.name}-xw{k}", ins=[], outs=[]
                        )
                        ev.engine = ins.engine
                        ev.sync_info = SyncInfo(on_wait=[w], on_update=[])
                        out.append(ev)
                    si.on_wait = waits[-1:]
                    ins.sync_info = si
                    changed = True
                out.append(ins)
            if changed:
                bb.instructions = out


def _wrap_idx(idx):
    """[n] -> [128, n/16] int16: idx[i] at [i%16, i//16], replicated x8."""
    a = np.asarray(idx, np.int16).reshape(-1, 16).T
    return np.ascontiguousarray(np.tile(a, (8, 1)))


def _pack_windows(edge_lists, kw_list):
    """edge_lists: per window, (srcrel, attr, dstrank) arrays.
    Returns srcrel [128, C], attr [128, C], dstrank flat [C*128] with
    position (chunk, partition) = edge chunk*128+partition, windows
    concatenated chunk-major; padding edges srcrel=0/attr=0/dst=0."""
    C = sum(kw_list)
    srcrel = np.zeros((128, C), np.float32)
    attr = np.zeros((128, C), np.float32)
    dstr = np.zeros(C * 128, np.int64)
    col = 0
    for (sr, at, dr), kw in zip(edge_lists, kw_list, strict=True):
        m = len(sr)
        b = np.zeros(kw * 128, np.float32)
        b[:m] = sr
        srcrel[:, col : col + kw] = b.reshape(kw, 128).T
        b = np.zeros(kw * 128, np.float32)
        b[:m] = at
        attr[:, col : col + kw] = b.reshape(kw, 128).T
        b = np.zeros(kw * 128, np.int64)
        b[:m] = dr
        dstr[col * 128 : (col + kw) * 128] = b
        col += kw
    return srcrel, attr, dstr


def _prep(fea, perm, eidx, eattr, natte, n, ncores):
    """Host-side sharding/index prep. Numeric compute stays on device."""
    N = int(n)
    K = perm.shape[0]
    R = N // ncores
    W = R // 128
    src = eidx[0].astype(np.int64)
    dst = eidx[1].astype(np.int64)
    attr = eattr.astype(np.float32)

    kept = np.zeros(N, bool)
    kept[perm] = True
    rank = np.zeros(N, np.int64)
    rank[perm] = np.arange(K)

    # prop edge list: kept-dst edges + self edges (attr=1) for kept nodes
    keep_e = kept[dst]
    psrc = np.concatenate([src[keep_e], perm.astype(np.int64)])
    pdst = np.concatenate([rank[dst[keep_e]], rank[perm]])
    pattr = np.concatenate([attr[keep_e], np.ones(K, np.float32)])

    def bucket(s, a, dr):
        """-> per (core, window) edge arrays + shared chunk counts."""
        g = s // 128  # global window id
        order = np.argsort(g, kind="stable")
        s, a, g = s[order], a[order], g[order]
        dr = dr[order] if dr is not None else None
        counts = np.bincount(g, minlength=W * ncores)
        kw = np.maximum(
            1, -(-counts.reshape(ncores, W).max(axis=0) // 128)
        )  # [W] shared chunk counts
        offs = np.concatenate([[0], np.cumsum(counts)])
        per_core = []
        for c in range(ncores):
            lists = []
            for w in range(W):
                gi = c * W + w
                sl = slice(offs[gi], offs[gi + 1])
                lists.append(
                    (
                        (s[sl] % 128).astype(np.float32),
                        a[sl],
                        dr[sl]
                        if dr is not None
                        else np.zeros(offs[gi + 1] - offs[gi], np.int64),
                    )
                )
            per_core.append(_pack_windows(lists, kw))
        return per_core, list(int(x) for x in kw)

    prop_per_core, kprop = bucket(psrc, pattr, pdst)

    # deg: row-major packing — row r's edge attrs along the free dim of
    # partition r%128 (one tensor_reduce computes all degrees)
    order = np.argsort(src, kind="stable")
    ssrc = src[order]
    sattr = attr[order]
    counts = np.bincount(ssrc, minlength=N)
    MD = max(4, int(-(-counts.max() // 4) * 4))
    starts = np.concatenate([[0], np.cumsum(counts)])
    pos = np.arange(len(ssrc)) - starts[ssrc]
    byrow = np.zeros((N, MD), np.float32)
    byrow[ssrc, pos] = sattr

    WG = N // 128
    dga_full = np.ascontiguousarray(byrow.reshape(WG, 128, MD).transpose(1, 0, 2))

    dt_prop = ml_dtypes.bfloat16 if PROP_BF16 else np.float32
    in_maps = []
    for c in range(ncores):
        psr, pat, pdr = prop_per_core[c]
        dga = (
            byrow[c * R : (c + 1) * R]
            .reshape(W, 128, MD)
            .transpose(1, 0, 2)
        )
        in_maps.append(
            {
                "dgaf": dga_full,
                "dga": np.ascontiguousarray(dga),
                "psr": psr.astype(dt_prop),
                "pat": pat.astype(dt_prop),
                "pix": _wrap_idx(pdr),
                "kix": _wrap_idx(perm.astype(np.int64)),
                "fea": np.ascontiguousarray(fea.astype(np.float32)),
                "nat": np.ascontiguousarray(natte.astype(np.float32)),
            }
        )
    return in_maps, MD, kprop, K, R, W, N


def _build(N, K, R, W, MD, kprop, skip=frozenset()):
    CProp = sum(kprop)
    KCH = K // 128
    DP = _dp()
    TPROP = BF16 if PROP_BF16 else F32

    nc = bacc.Bacc(
        "TRN2",
        target_bir_lowering=False,
        debug=False,
        num_devices=NCORES,
        num_swdge_queues=NQUEUES,
    )

    WG = N // 128
    dgaf_d = nc.dram_tensor("dgaf", [128, WG, MD], F32, kind="ExternalInput")
    dga_d = nc.dram_tensor("dga", [128, W, MD], F32, kind="ExternalInput")
    psr_d = nc.dram_tensor("psr", [128, CProp], TPROP, kind="ExternalInput")
    pat_d = nc.dram_tensor("pat", [128, CProp], TPROP, kind="ExternalInput")
    pix_d = nc.dram_tensor("pix", [128, CProp * 8], I16, kind="ExternalInput")
    kix_d = nc.dram_tensor("kix", [128, K // 16], I16, kind="ExternalInput")
    fea_d = nc.dram_tensor("fea", [K, D_FEAT], F32, kind="ExternalInput")
    nat_d = nc.dram_tensor("nat", [K, 1], F32, kind="ExternalInput")
    out_d = nc.dram_tensor("out", [R, DO], F32, kind="ExternalOutput")

    dpf_d = nc.dram_tensor("dpf", [N, 64], F32)  # locally built dis table
    zsk_d = nc.dram_tensor("zsk", [K, DP], TPROP)  # gather table

    with tile.TileContext(nc) as tc:
        with (
            tc.tile_pool(name="cst", bufs=1) as cst,
            tc.tile_pool(name="sprop", bufs=W) as sprop,
            tc.tile_pool(name="gwin", bufs=4) as gwin,
            tc.tile_pool(name="ps", bufs=2, space="PSUM") as ps,
        ):
            nc.gpsimd.load_library(mlp)

            iota_prop = cst.tile([128, 128], TPROP)
            nc.gpsimd.iota(
                iota_prop[:], [[1, 128]], channel_multiplier=0,
                allow_small_or_imprecise_dtypes=True,
            )

            # pad columns of the two tables are gathered but sliced away
            # before any arithmetic; zero them only for CoreSim's
            # uninitialized-read checker (SIM_SAFE=1), not on HW
            dpt = cst.tile([128, WG, 64], F32)
            if _os.environ.get("SIM_SAFE", "0") == "1":
                nc.vector.memset(dpt[:].rearrange("p a d -> p (a d)"), 0.0)

            dgaf_t = cst.tile([128, WG, MD], F32)
            dga_t = cst.tile([128, W, MD], F32)
            psr_t = cst.tile([128, CProp], TPROP)
            pat_t = cst.tile([128, CProp], TPROP)
            pix_t = cst.tile([128, CProp * 8], I16)
            kix_t = cst.tile([128, K // 16], I16)
            nc.sync.dma_start(dgaf_t[:], dgaf_d[:])
            nc.sync.dma_start(dga_t[:], dga_d[:])
            nc.sync.dma_start(kix_t[:], kix_d[:])
            nc.sync.dma_start(psr_t[:], psr_d[:])
            nc.sync.dma_start(pat_t[:], pat_d[:])

            fea_t = cst.tile([128, KCH, D_FEAT], F32)
            nc.sync.dma_start(fea_t[:], fea_d[:].rearrange("(a p) d -> p a d", p=128))
            nat_t = cst.tile([128, KCH], F32)
            nc.sync.dma_start(
                nat_t[:], nat_d[:].rearrange("(a p) one -> p (a one)", p=128)
            )
            nc.sync.dma_start(pix_t[:], pix_d[:])

            # ---- phase 1+2: degrees for the WHOLE graph, replicated on every
            # core (1.3MB input + one reduce beats a ~70us-fixed-latency
            # AllGather), plus a local copy for the final scaling ----
            def rsqrt_chain(x_wide, width):
                degp = cst.tile([128, width], F32, tag=f"degp{width}")
                nc.vector.tensor_scalar_add(degp[:], x_wide, 1.0)
                dis = cst.tile([128, width], F32, tag=f"dis{width}")
                nc.vector.reciprocal(dis[:], degp[:])
                nc.scalar.activation(
                    dis[:], dis[:], mybir.ActivationFunctionType.Sqrt
                )
                tmp = cst.tile([128, width], F32, tag=f"tmp{width}")
                for _ in range(2):
                    nc.vector.tensor_mul(tmp[:], dis[:], dis[:])
                    nc.vector.tensor_mul(tmp[:], tmp[:], degp[:])
                    nc.vector.tensor_scalar(
                        tmp[:], tmp[:], -0.5, 1.5,
                        mybir.AluOpType.mult, mybir.AluOpType.add,
                    )
                    nc.vector.tensor_mul(dis[:], dis[:], tmp[:])
                return dis

            degf_t = cst.tile([128, WG], F32)
            nc.vector.tensor_reduce(
                degf_t[:].unsqueeze(-1), dgaf_t[:], mybir.AxisListType.X,
                mybir.AluOpType.add,
            )
            disf_t = rsqrt_chain(degf_t[:], WG)

            deg_t = cst.tile([128, W], F32)
            nc.vector.tensor_reduce(
                deg_t[:].unsqueeze(-1), dga_t[:], mybir.AxisListType.X,
                mybir.AluOpType.add,
            )
            dis_t = rsqrt_chain(deg_t[:], W)

            # dis table [N, 64] written locally; only col 0 is ever gathered
            nc.vector.tensor_copy(dpt[:, :, 0:1], disf_t[:].unsqueeze(-1))
            nc.scalar.dma_start(dpf_d[:].rearrange("(g p) e -> p g e", p=128), dpt[:])

            # ---- prop one-hots: emitted after the collective so the tiny
            # deg->dis chain wins the Vector stream; these overlap the
            # collective wait and the dis-table gather ----
            s_props = []
            off = 0
            for w in range(W):
                kw = kprop[w]
                S = sprop.tile([128, kw, 128], TPROP, tag="S")
                nc.vector.tensor_tensor(
                    S[:],
                    iota_prop[:].unsqueeze(1).broadcast_to([128, kw, 128]),
                    psr_t[:, off : off + kw].unsqueeze(-1).broadcast_to([128, kw, 128]),
                    mybir.AluOpType.is_equal,
                )
                nc.vector.tensor_tensor(
                    S[:],
                    S[:],
                    pat_t[:, off : off + kw].unsqueeze(-1).broadcast_to([128, kw, 128]),
                    mybir.AluOpType.mult,
                )
                s_props.append(S)
                off += kw

            # ---- phase 3: zs_k table ----
            disk = cst.tile([128, KCH, 64], F32)
            if "kgather" in skip:
                nc.vector.memset(disk[:].rearrange("p a d -> p (a d)"), 0.25)
            else:
                kq = KCH // NQUEUES
                for q in range(NQUEUES):
                    nc.gpsimd.dma_gather(
                        disk[:, q * kq : (q + 1) * kq, :], dpf_d[:],
                        kix_t[:, q * kq * 8 : (q + 1) * kq * 8],
                        kq * 128, kq * 128, 64,
                        single_packet=False, queue_num=q,
                    )
            zsk_t = cst.tile([128, KCH, DP], TPROP)
            if _os.environ.get("SIM_SAFE", "0") == "1":
                nc.vector.memset(zsk_t[:].rearrange("p a d -> p (a d)"), 0.0)
            nc.vector.tensor_tensor(
                zsk_t[:, :, 0:D_FEAT],
                fea_t[:],
                disk[:, :, 0:1].broadcast_to([128, KCH, D_FEAT]),
                mybir.AluOpType.mult,
            )
            nc.vector.tensor_tensor(
                zsk_t[:, :, D_FEAT : D_FEAT + 1],
                nat_t[:].unsqueeze(-1),
                disk[:, :, 0:1],
                mybir.AluOpType.mult,
            )
            nc.scalar.dma_start(zsk_d[:].rearrange("(a p) d -> p a d", p=128), zsk_t[:])

            # ---- phase 4: propagate per window ----
            off = 0
            for w in range(W):
                kw = kprop[w]
                S = s_props[w]
                g = gwin.tile([128, kw, DP], TPROP, tag="g")
                if "ggather" in skip:
                    nc.vector.memset(g[:].rearrange("p a d -> p (a d)"), 0.5)
                else:
                    nc.gpsimd.dma_gather(
                        g[:], zsk_d[:], pix_t[:, off * 8 : (off + kw) * 8],
                        kw * 128, kw * 128, DP, single_packet=False,
                        queue_num=w % NQUEUES,
                    )
                py = ps.tile([128, DO], F32, tag="py")
                for k in range(kw):
                    nc.tensor.matmul(
                        py[:],
                        S[:, k, :],
                        g[:, k, 0:DO],
                        start=(k == 0),
                        stop=(k == kw - 1),
                    )
                ot = gwin.tile([128, DO], F32, tag="ot")
                nc.vector.tensor_scalar(
                    ot[:], py[:], dis_t[:, w : w + 1], None, mybir.AluOpType.mult
                )
                nc.sync.dma_start(
                    out_d[:].rearrange("(w p) d -> p w d", p=128)[:, w, :], ot[:]
                )
                off += kw

    nc.compile()
    return nc


_CACHE = {}


def kernel(fea, perm, encoder_edge_index, encoder_edge_attr, node_atte_coffe, node_num):
    fea = np.asarray(fea)
    perm = np.asarray(perm)
    eidx = np.asarray(encoder_edge_index)
    eattr = np.asarray(encoder_edge_attr)
    natte = np.asarray(node_atte_coffe)
    n = int(node_num)

    in_maps, MD, kprop, K, R, W, N = _prep(fea, perm, eidx, eattr, natte, n, NCORES)

    key = (N, K, MD, tuple(kprop))
    if key not in _CACHE:
        nc = _build(N, K, R, W, MD, kprop)
        _split_multi_waits(nc)
        _CACHE[key] = nc
    nc = _CACHE[key]

    res = run_bass_kernel_spmd(nc, in_maps, core_ids=list(range(NCORES)))
    full = np.concatenate([res.results[c]["out"] for c in range(NCORES)], axis=0)
    return full[:, :D_FEAT], full[:, D_FEAT:DO]


# revision 30
# speedup vs baseline: 1.3858x; 1.0347x over previous
"""Trainium2 Bass kernel for nn_DiffusioUnpool (GNN message passing).

Math: out = P @ z where P = D^-1/2 (A_e + I) D^-1/2, z = scatter(fea|atte, perm),
rewritten as segment-sum SpMM:
    deg[i]  = 1 + sum_{e: src=i} attr[e]
    dis     = rsqrt(deg)
    zs_k[k] = dis[perm[k]] * [fea[k], atte[k], 0pad]      (compact kept-node table)
    out[i]  = dis[i] * sum_{e: src=i, kept(dst)} attr[e] * zs_k[rank(dst)]
(self-loops folded in as edges with attr=1 and dst=i for kept i)

Sharding: row-shard across 8 cores (core c owns rows [1024c, 1024c+1024));
edges bucketed by src owner. Per core: all-graph degrees via one tensor_reduce
over a replicated row-major attr layout (an ncfw AllGather measured ~70us
FIXED latency regardless of payload, so replicating 1.3MB + one reduce is far
cheaper than communicating), locally built dis gather-table, dma_gather of
kept-node rows (4 SWDGE queues, single_packet=False), one-hot matmul
segment-sum into PSUM per 128-row window, final dis scaling.

dma_scatter_add is NOT used for accumulation: measured on HW, duplicate
indices within one call lose updates (only ~2 of 8 same-position duplicate
contributions land), so edge reductions go through PE one-hot matmuls and
the degree reduction uses a row-major attr layout + one tensor_reduce.

Numerics: f32 end-to-end (measured rel err ~3e-7 vs the f32 reference);
dis = reciprocal + sqrt + two Newton steps. A bf16 propagation path exists
(PROP_BF16=1, rel err ~2e-3) but is not faster: the span is bound by the
gather descriptor generation and serial chain latency more than compute
bytes (bf16 measured 131us vs 146us f32; not worth the error).
"""
import os as _os

import ml_dtypes
import numpy as np

import concourse.bacc as bacc
import concourse.mybir as mybir
import concourse.tile as tile
from concourse.bass_utils import run_bass_kernel_spmd
from concourse.library_config import mlp
from bass_rust import SyncInfo

F32 = mybir.dt.float32
BF16 = mybir.dt.bfloat16
I16 = mybir.dt.int16

NCORES = 8
NQUEUES = int(_os.environ.get("NQUEUES", "4"))
PROP_BF16 = _os.environ.get("PROP_BF16", "0") == "1"
D_FEAT = 128
DO = 129  # meaningful output row width: fea(128) | atte(1)


def _dp():
    # gatherable zs_k row width: 512B in bf16, 768B in f32 (256B-multiple rule)
    return 256 if PROP_BF16 else 192


def _split_multi_waits(nc):
    """This walrus build only encodes one sem-wait per instruction; hoist
    extras into wait-only EventSemaphore instructions just before."""
    for f in nc.m.functions:
        for bb in f.blocks:
            out = []
            changed = False
            for ins in bb.instructions:
                si = ins.sync_info
                if si is not None and si.on_wait is not None and len(si.on_wait) > 1:
                    waits = list(si.on_wait)
                    for k, w in enumerate(waits[:-1]):
                        ev = mybir.InstEventSemaphore(
                            name=f"{ins.name}-xw{k}", ins=[], outs=[]
                        )
                        ev.engine = ins.engine
                        ev.sync_info = SyncInfo(on_wait=[w], on_update=[])
                        out.append(ev)
                    si.on_wait = waits[-1:]
                    ins.sync_info = si
                    changed = True
                out.append(ins)
            if changed:
                bb.instructions = out


def _wrap_idx(idx):
    """[n] -> [128, n/16] int16: idx[i] at [i%16, i//16], replicated x8."""
    a = np.asarray(idx, np.int16).reshape(-1, 16).T
    return np.ascontiguousarray(np.tile(a, (8, 1)))


def _pack_windows(edge_lists, kw_list):
    """edge_lists: per window, (srcrel, attr, dstrank) arrays.
    Returns srcrel [128, C], attr [128, C], dstrank flat [C*128] with
    position (chunk, partition) = edge chunk*128+partition, windows
    concatenated chunk-major; padding edges srcrel=0/attr=0/dst=0."""
    C = sum(kw_list)
    srcrel = np.zeros((128, C), np.float32)
    attr = np.zeros((128, C), np.float32)
    dstr = np.zeros(C * 128, np.int64)
    col = 0
    for (sr, at, dr), kw in zip(edge_lists, kw_list, strict=True):
        m = len(sr)
        b = np.zeros(kw * 128, np.float32)
        b[:m] = sr
        srcrel[:, col : col + kw] = b.reshape(kw, 128).T
        b = np.zeros(kw * 128, np.float32)
        b[:m] = at
        attr[:, col : col + kw] = b.reshape(kw, 128).T
        b = np.zeros(kw * 128, np.int64)
        b[:m] = dr
        dstr[col * 128 : (col + kw) * 128] = b
        col += kw
    return srcrel, attr, dstr


def _prep(fea, perm, eidx, eattr, natte, n, ncores):
    """Host-side sharding/index prep. Numeric compute stays on device."""
    N = int(n)
    K = perm.shape[0]
    R = N // ncores
    W = R // 128
    src = eidx[0].astype(np.int64)
    dst = eidx[1].astype(np.int64)
    attr = eattr.astype(np.float32)

    kept = np.zeros(N, bool)
    kept[perm] = True
    rank = np.zeros(N, np.int64)
    rank[perm] = np.arange(K)

    # prop edge list: kept-dst edges + self edges (attr=1) for kept nodes
    keep_e = kept[dst]
    psrc = np.concatenate([src[keep_e], perm.astype(np.int64)])
    pdst = np.concatenate([rank[dst[keep_e]], rank[perm]])
    pattr = np.concatenate([attr[keep_e], np.ones(K, np.float32)])

    def bucket(s, a, dr):
        """-> per (core, window) edge arrays + shared chunk counts."""
        g = s // 128  # global window id
        order = np.argsort(g, kind="stable")
        s, a, g = s[order], a[order], g[order]
        dr = dr[order] if dr is not None else None
        counts = np.bincount(g, minlength=W * ncores)
        kw = np.maximum(
            1, -(-counts.reshape(ncores, W).max(axis=0) // 128)
        )  # [W] shared chunk counts
        offs = np.concatenate([[0], np.cumsum(counts)])
        per_core = []
        for c in range(ncores):
            lists = []
            for w in range(W):
                gi = c * W + w
                sl = slice(offs[gi], offs[gi + 1])
                lists.append(
                    (
                        (s[sl] % 128).astype(np.float32),
                        a[sl],
                        dr[sl]
                        if dr is not None
                        else np.zeros(offs[gi + 1] - offs[gi], np.int64),
                    )
                )
            per_core.append(_pack_windows(lists, kw))
        return per_core, list(int(x) for x in kw)

    prop_per_core, kprop = bucket(psrc, pattr, pdst)

    # deg: row-major packing — row r's edge attrs along the free dim of
    # partition r%128 (one tensor_reduce computes all degrees)
    order = np.argsort(src, kind="stable")
    ssrc = src[order]
    sattr = attr[order]
    counts = np.bincount(ssrc, minlength=N)
    MD = max(4, int(-(-counts.max() // 4) * 4))
    starts = np.concatenate([[0], np.cumsum(counts)])
    pos = np.arange(len(ssrc)) - starts[ssrc]
    byrow = np.zeros((N, MD), np.float32)
    byrow[ssrc, pos] = sattr

    WG = N // 128
    dga_full = np.ascontiguousarray(byrow.reshape(WG, 128, MD).transpose(1, 0, 2))

    dt_prop = ml_dtypes.bfloat16 if PROP_BF16 else np.float32
    in_maps = []
    for c in range(ncores):
        psr, pat, pdr = prop_per_core[c]
        dga = (
            byrow[c * R : (c + 1) * R]
            .reshape(W, 128, MD)
            .transpose(1, 0, 2)
        )
        in_maps.append(
            {
                "dgaf": dga_full,
                "dga": np.ascontiguousarray(dga),
                "psr": psr.astype(dt_prop),
                "pat": pat.astype(dt_prop),
                "pix": _wrap_idx(pdr),
                "kix": _wrap_idx(perm.astype(np.int64)),
                "fea": np.ascontiguousarray(fea.astype(np.float32)),
                "nat": np.ascontiguousarray(natte.astype(np.float32)),
            }
        )
    return in_maps, MD, kprop, K, R, W, N


def _build(N, K, R, W, MD, kprop, skip=frozenset()):
    CProp = sum(kprop)
    KCH = K // 128
    DP = _dp()
    TPROP = BF16 if PROP_BF16 else F32

    nc = bacc.Bacc(
        "TRN2",
        target_bir_lowering=False,
        debug=False,
        num_devices=NCORES,
        num_swdge_queues=NQUEUES,
    )

    WG = N // 128
    dgaf_d = nc.dram_tensor("dgaf", [128, WG, MD], F32, kind="ExternalInput")
    dga_d = nc.dram_tensor("dga", [128, W, MD], F32, kind="ExternalInput")
    psr_d = nc.dram_tensor("psr", [128, CProp], TPROP, kind="ExternalInput")
    pat_d = nc.dram_tensor("pat", [128, CProp], TPROP, kind="ExternalInput")
    pix_d = nc.dram_tensor("pix", [128, CProp * 8], I16, kind="ExternalInput")
    kix_d = nc.dram_tensor("kix", [128, K // 16], I16, kind="ExternalInput")
    fea_d = nc.dram_tensor("fea", [K, D_FEAT], F32, kind="ExternalInput")
    nat_d = nc.dram_tensor("nat", [K, 1], F32, kind="ExternalInput")
    out_d = nc.dram_tensor("out", [R, DO], F32, kind="ExternalOutput")

    dpf_d = nc.dram_tensor("dpf", [N, 64], F32)  # locally built dis table
    zsk_d = nc.dram_tensor("zsk", [K, DP], TPROP)  # gather table

    with tile.TileContext(nc) as tc:
        with (
            tc.tile_pool(name="cst", bufs=1) as cst,
            tc.tile_pool(name="sprop", bufs=W) as sprop,
            tc.tile_pool(name="gwin", bufs=6) as gwin,
            tc.tile_pool(name="ps", bufs=2, space="PSUM") as ps,
        ):
            nc.gpsimd.load_library(mlp)

            iota_prop = cst.tile([128, 128], TPROP)
            nc.gpsimd.iota(
                iota_prop[:], [[1, 128]], channel_multiplier=0,
                allow_small_or_imprecise_dtypes=True,
            )

            # pad columns of the two tables are gathered but sliced away
            # before any arithmetic; zero them only for CoreSim's
            # uninitialized-read checker (SIM_SAFE=1), not on HW
            dpt = cst.tile([128, WG, 64], F32)
            if _os.environ.get("SIM_SAFE", "0") == "1":
                nc.vector.memset(dpt[:].rearrange("p a d -> p (a d)"), 0.0)

            dgaf_t = cst.tile([128, WG, MD], F32)
            dga_t = cst.tile([128, W, MD], F32)
            psr_t = cst.tile([128, CProp], TPROP)
            pat_t = cst.tile([128, CProp], TPROP)
            pix_t = cst.tile([128, CProp * 8], I16)
            kix_t = cst.tile([128, K // 16], I16)
            nc.sync.dma_start(dgaf_t[:], dgaf_d[:])
            nc.sync.dma_start(dga_t[:], dga_d[:])
            nc.sync.dma_start(kix_t[:], kix_d[:])
            nc.sync.dma_start(psr_t[:], psr_d[:])
            nc.sync.dma_start(pat_t[:], pat_d[:])

            fea_t = cst.tile([128, KCH, D_FEAT], F32)
            nc.sync.dma_start(fea_t[:], fea_d[:].rearrange("(a p) d -> p a d", p=128))
            nat_t = cst.tile([128, KCH], F32)
            nc.sync.dma_start(
                nat_t[:], nat_d[:].rearrange("(a p) one -> p (a one)", p=128)
            )
            nc.sync.dma_start(pix_t[:], pix_d[:])

            # ---- phase 1+2: degrees for the WHOLE graph, replicated on every
            # core (1.3MB input + one reduce beats a ~70us-fixed-latency
            # AllGather), plus a local copy for the final scaling ----
            def rsqrt_chain(x_wide, width):
                degp = cst.tile([128, width], F32, tag=f"degp{width}")
                nc.vector.tensor_scalar_add(degp[:], x_wide, 1.0)
                dis = cst.tile([128, width], F32, tag=f"dis{width}")
                nc.vector.reciprocal(dis[:], degp[:])
                nc.scalar.activation(
                    dis[:], dis[:], mybir.ActivationFunctionType.Sqrt
                )
                tmp = cst.tile([128, width], F32, tag=f"tmp{width}")
                for _ in range(2):
                    nc.vector.tensor_mul(tmp[:], dis[:], dis[:])
                    nc.vector.tensor_mul(tmp[:], tmp[:], degp[:])
                    nc.vector.tensor_scalar(
                        tmp[:], tmp[:], -0.5, 1.5,
                        mybir.AluOpType.mult, mybir.AluOpType.add,
                    )
                    nc.vector.tensor_mul(dis[:], dis[:], tmp[:])
                return dis

            degf_t = cst.tile([128, WG], F32)
            nc.vector.tensor_reduce(
                degf_t[:].unsqueeze(-1), dgaf_t[:], mybir.AxisListType.X,
                mybir.AluOpType.add,
            )
            disf_t = rsqrt_chain(degf_t[:], WG)

            deg_t = cst.tile([128, W], F32)
            nc.vector.tensor_reduce(
                deg_t[:].unsqueeze(-1), dga_t[:], mybir.AxisListType.X,
                mybir.AluOpType.add,
            )
            dis_t = rsqrt_chain(deg_t[:], W)

            # dis table [N, 64] written locally; only col 0 is ever gathered
            nc.vector.tensor_copy(dpt[:, :, 0:1], disf_t[:].unsqueeze(-1))
            nc.scalar.dma_start(dpf_d[:].rearrange("(g p) e -> p g e", p=128), dpt[:])

            # ---- prop one-hots: emitted after the collective so the tiny
            # deg->dis chain wins the Vector stream; these overlap the
            # collective wait and the dis-table gather ----
            s_props = []
            off = 0
            for w in range(W):
                kw = kprop[w]
                S = sprop.tile([128, kw, 128], TPROP, tag="S")
                nc.vector.tensor_tensor(
                    S[:],
                    iota_prop[:].unsqueeze(1).broadcast_to([128, kw, 128]),
                    psr_t[:, off : off + kw].unsqueeze(-1).broadcast_to([128, kw, 128]),
                    mybir.AluOpType.is_equal,
                )
                nc.vector.tensor_tensor(
                    S[:],
                    S[:],
                    pat_t[:, off : off + kw].unsqueeze(-1).broadcast_to([128, kw, 128]),
                    mybir.AluOpType.mult,
                )
                s_props.append(S)
                off += kw

            # ---- phase 3: zs_k table ----
            disk = cst.tile([128, KCH, 64], F32)
            if "kgather" in skip:
                nc.vector.memset(disk[:].rearrange("p a d -> p (a d)"), 0.25)
            else:
                kq = KCH // NQUEUES
                for q in range(NQUEUES):
                    nc.gpsimd.dma_gather(
                        disk[:, q * kq : (q + 1) * kq, :], dpf_d[:],
                        kix_t[:, q * kq * 8 : (q + 1) * kq * 8],
                        kq * 128, kq * 128, 64,
                        single_packet=False, queue_num=q,
                    )
            zsk_t = cst.tile([128, KCH, DP], TPROP)
            if _os.environ.get("SIM_SAFE", "0") == "1":
                nc.vector.memset(zsk_t[:].rearrange("p a d -> p (a d)"), 0.0)
            nc.vector.tensor_tensor(
                zsk_t[:, :, 0:D_FEAT],
                fea_t[:],
                disk[:, :, 0:1].broadcast_to([128, KCH, D_FEAT]),
                mybir.AluOpType.mult,
            )
            nc.vector.tensor_tensor(
                zsk_t[:, :, D_FEAT : D_FEAT + 1],
                nat_t[:].unsqueeze(-1),
                disk[:, :, 0:1],
                mybir.AluOpType.mult,
            )
            nc.scalar.dma_start(zsk_d[:].rearrange("(a p) d -> p a d", p=128), zsk_t[:])

            # ---- phase 4: propagate per window ----
            off = 0
            for w in range(W):
                kw = kprop[w]
                S = s_props[w]
                g = gwin.tile([128, kw, DP], TPROP, tag="g")
                if "ggather" in skip:
                    nc.vector.memset(g[:].rearrange("p a d -> p (a d)"), 0.5)
                else:
                    nc.gpsimd.dma_gather(
                        g[:], zsk_d[:], pix_t[:, off * 8 : (off + kw) * 8],
                        kw * 128, kw * 128, DP, single_packet=False,
                        queue_num=w % NQUEUES,
                    )
                py = ps.tile([128, DO], F32, tag="py")
                for k in range(kw):
                    nc.tensor.matmul(
                        py[:],
                        S[:, k, :],
                        g[:, k, 0:DO],
                        start=(k == 0),
                        stop=(k == kw - 1),
                    )
                ot = gwin.tile([128, DO], F32, tag="ot")
                nc.vector.tensor_scalar(
                    ot[:], py[:], dis_t[:, w : w + 1], None, mybir.AluOpType.mult
                )
                nc.sync.dma_start(
                    out_d[:].rearrange("(w p) d -> p w d", p=128)[:, w, :], ot[:]
                )
                off += kw

    nc.compile()
    return nc


_CACHE = {}


def kernel(fea, perm, encoder_edge_index, encoder_edge_attr, node_atte_coffe, node_num):
    fea = np.asarray(fea)
    perm = np.asarray(perm)
    eidx = np.asarray(encoder_edge_index)
    eattr = np.asarray(encoder_edge_attr)
    natte = np.asarray(node_atte_coffe)
    n = int(node_num)

    in_maps, MD, kprop, K, R, W, N = _prep(fea, perm, eidx, eattr, natte, n, NCORES)

    key = (N, K, MD, tuple(kprop))
    if key not in _CACHE:
        nc = _build(N, K, R, W, MD, kprop)
        _split_multi_waits(nc)
        _CACHE[key] = nc
    nc = _CACHE[key]

    res = run_bass_kernel_spmd(nc, in_maps, core_ids=list(range(NCORES)))
    full = np.concatenate([res.results[c]["out"] for c in range(NCORES)], axis=0)
    return full[:, :D_FEAT], full[:, D_FEAT:DO]
